# revision 1
# baseline (speedup 1.0000x reference)
"""GATv2 2-layer GNN on 8 Trainium2 NeuronCores (Bass/Tile).

Sharding: dst-range (6250 dsts/core), edges dst-sorted into 49 windows of
128 dsts. Per-edge endpoint rows are fetched with dma_gather from f16 tables
(512B rows, int16 indices -> tables split at row 25088 into A/B halves).
The |att|-fold plus a +-1 sign mask turns sum_c att_c*lrelu(m_c) into
lrelu + masked reduce. Segment softmax skips max-subtraction (e stays in
[-7, 7] for this model; exp in f16 is safe). Segment sums run on the PE as
0/1-indicator matmuls accumulated into one PSUM tile per window. Layer 2
reuses the same index streams on its own tables. The program is
SPMD-uniform: per-window tile counts are maxed over cores; cores pad with
idx=0 / seg=128 slots which contribute zero.
"""
import sys

sys.path.insert(0, "/opt/trn_rl_repo")

import numpy as np

N = 50000
IN, HID, H, OUT = 128, 64, 4, 64
SLOPE = 0.2
NC = 8
NLOC = N // NC            # 6250
NPAD = 6272               # 49*128
NWIN = NPAD // 128        # 49
NROWS = NPAD * NC         # 50176
SPLIT = NROWS // 2        # 25088
GW = 2                    # windows per gather group
STREAM_MODE = False       # debug: replace gathers with sequential streams
CALL_CAP = 12             # max tiles (x128 idx) per dma_gather call

_cache = {}


def _wrap16(stream):
    n = len(stream)
    a = np.zeros((16, n // 16), np.int16)
    a[np.arange(n) % 16, np.arange(n) // 16] = stream
    return np.tile(a, (8, 1))


def _host_metadata(edge_index):
    src = np.concatenate([np.asarray(edge_index[0], np.int64), np.arange(N)])
    dst = np.concatenate([np.asarray(edge_index[1], np.int64), np.arange(N)])
    srow = (src // NLOC) * NPAD + (src % NLOC)
    dcore = dst // NLOC
    dloc = dst % NLOC

    per_cw = [[None] * NWIN for _ in range(NC)]
    for c in range(NC):
        m = dcore == c
        sr, dl = srow[m], dloc[m]
        o = np.argsort(dl, kind="stable")
        sr, dl = sr[o], dl[o]
        wi = dl // 128
        for w in range(NWIN):
            ws = wi == w
            s_w, d_w = sr[ws], dl[ws] - w * 128
            a = s_w < SPLIT
            per_cw[c][w] = (s_w[a], d_w[a], s_w[~a] - SPLIT, d_w[~a])

    TA = [max((len(per_cw[c][w][0]) + 127) // 128 for c in range(NC))
          for w in range(NWIN)]
    TB = [max((len(per_cw[c][w][2]) + 127) // 128 for c in range(NC))
          for w in range(NWIN)]

    groups = []
    w = 0
    ti = 0
    while w < NWIN:
        ws = list(range(w, min(w + GW, NWIN)))
        na = sum(TA[x] for x in ws)
        nb = sum(TB[x] for x in ws)
        slots = [(x, "A", i) for x in ws for i in range(TA[x])] + \
                [(x, "B", i) for x in ws for i in range(TB[x])]
        t = len(slots)
        calls = []
        for kind, lo, hi in (("A", 0, na), ("B", na, na + nb), ("X", 0, t)):
            s0 = lo
            while s0 < hi:
                n = min(CALL_CAP, hi - s0)
                calls.append((kind, s0, n))
                s0 += n
        groups.append(dict(ws=ws, na=na, nb=nb, slots=slots, calls=calls,
                           cols=sum(n * 8 for _, _, n in calls), ti=ti))
        ti += t
        w += GW
    tot_tiles = ti

    seg_all = np.full((NC, 128, tot_tiles), 128, np.int16)
    idx_all = [np.zeros((NC, 128, g["cols"]), np.int16) for g in groups]
    for gi, g in enumerate(groups):
        nslot = len(g["slots"])
        for c in range(NC):
            slot_idx = np.zeros((nslot, 128), np.int64)
            slot_xr = np.zeros((nslot, 128), np.int64)
            for si, (w, kind, k) in enumerate(g["slots"]):
                sa, da, sb, db = per_cw[c][w]
                s_, d_ = (sa, da) if kind == "A" else (sb, db)
                iv = s_[k * 128:(k + 1) * 128]
                dv = d_[k * 128:(k + 1) * 128]
                n = len(iv)
                slot_idx[si, :n] = iv
                slot_xr[si, :n] = dv + w * 128
                seg_all[c, :n, g["ti"] + si] = dv
            co = 0
            for kind, s0, ntc in g["calls"]:
                arr = slot_xr if kind == "X" else slot_idx
                idx_all[gi][c, :, co:co + ntc * 8] = _wrap16(
                    arr[s0:s0 + ntc].reshape(-1))
                co += ntc * 8
    return groups, seg_all, idx_all, tot_tiles


def _build(groups, tot_tiles, rep=1):
    import concourse.bacc as bacc
    import concourse.mybir as mybir
    import concourse.tile as tile
    from concourse import library_config

    f16, f32, i16 = mybir.dt.float16, mybir.dt.float32, mybir.dt.int16
    A = mybir.AluOpType

    nc = bacc.Bacc("TRN2", target_bir_lowering=False, debug=False,
                   num_devices=NC)
    d = {}
    for name, shape, dt in [
            ("xT", [128, NPAD], f32), ("W1", [128, 512], f32),
            ("b1", [1, 512], f32), ("W2", [128, 256], f16),
            ("b2", [1, 128], f16), ("sg1", [128, 256], f16),
            ("sg2", [128, 64], f16), ("rc1", [128, 256], f16),
            ("rc2", [128, 64], f32), ("ba1", [128, 256], f16),
            ("ba2", [128, 64], f32), ("iot", [128, 128], i16),
            ("seg", [128, tot_tiles], i16)]:
        d[name] = nc.dram_tensor(name, shape, dt, kind="ExternalInput")
    idx_d = [nc.dram_tensor(f"idx{gi}", [128, g["cols"]], i16,
                            kind="ExternalInput")
             for gi, g in enumerate(groups)]
    out_d = nc.dram_tensor("out", [NPAD, 64], f32, kind="ExternalOutput")

    with tile.TileContext(nc) as tc:
        with (
            tc.tile_pool(name="const", bufs=1) as cp,
            tc.tile_pool(name="dram", bufs=1, space="DRAM") as dp,
        ):
            nc.gpsimd.load_library(library_config.mlp)
            T = {"out_d": out_d, "dp": dp, "idx_d": idx_d}
            for name in d:
                tl = cp.tile(list(d[name].shape), d[name].dtype, tag=name)
                nc.sync.dma_start(tl[:], d[name][:, :])
                T[name] = tl
            ones = cp.tile([1, 128], f32)
            nc.vector.memset(ones[:], 1.0)
            ones16 = cp.tile([1, 128], f16)
            nc.vector.memset(ones16[:], 1.0)
            T["ones"], T["ones16"] = ones, ones16

            for r in range(rep):
                _one_pass(nc, tc, mybir, groups, T)
    nc.compile()
    return nc


def _one_pass(nc, tc, mybir, groups, T):
    f16, f32 = mybir.dt.float16, mybir.dt.float32
    A = mybir.AluOpType
    dp = T["dp"]

    xl_loc = dp.tile([NPAD, 256], f16, tag="xl_loc")
    xr_loc = dp.tile([NPAD, 256], f16, tag="xr_loc")
    xl_full = dp.tile([NROWS, 256], f16, tag="xl_full")
    l2_loc = dp.tile([NPAD, 128], f16, tag="l2_loc")
    r2_loc = dp.tile([NPAD, 128], f16, tag="r2_loc")
    l2_full = dp.tile([NROWS, 128], f16, tag="l2_full")

    # ---- node phase 1: att-folded xl/xr tables for local nodes ----
    with (
        tc.tile_pool(name="n1", bufs=3) as n1,
        tc.tile_pool(name="n1p", bufs=2, space="PSUM") as n1p,
    ):
        for t in range(NWIN):
            ps = n1p.tile([128, 512], f32, tag="ps")
            nc.tensor.matmul(ps[:], T["ones"][:], T["b1"][:],
                             start=True, stop=False, skip_group_check=True)
            nc.tensor.matmul(ps[:], T["xT"][:, t * 128:(t + 1) * 128],
                             T["W1"][:], start=False, stop=True,
                             skip_group_check=True)
            v = n1.tile([128, 512], f16, tag="v")
            nc.scalar.copy(v[:], ps[:])
            nc.sync.dma_start(xl_loc[t * 128:(t + 1) * 128, :], v[:, 0:256])
            nc.sync.dma_start(xr_loc[t * 128:(t + 1) * 128, :], v[:, 256:512])

    nc.gpsimd.collective_compute(
        "AllGather", A.bypass, replica_groups=[list(range(NC))],
        ins=[xl_loc[:].opt()], outs=[xl_full[:].opt()])

    # ---- edge phase 1 -> h rows in DRAM ----
    h_dram = dp.tile([NPAD, 256], f16, tag="h_dram")
    if True:
        with (
            tc.tile_pool(name="e1", bufs=2) as e1,
            tc.tile_pool(name="e1s", bufs=1) as e1s,
            tc.tile_pool(name="e1p", bufs=4, space="PSUM") as e1p,
        ):
            _edge_layer(nc, tc, mybir, groups, T, e1, e1s, e1p,
                        layer=1, src_tab=xl_full, xr_tab=xr_loc, h=h_dram)

        # ---- node phase 2: hl2/hr2 tables ----
        with (
            tc.tile_pool(name="hT", bufs=1) as hTp,
            tc.tile_pool(name="n2", bufs=3) as n2,
            tc.tile_pool(name="n2p", bufs=2, space="PSUM") as n2p,
        ):
            hT = hTp.tile([128, 2, NPAD], f16)
            for k in range(2):
                nc.sync.dma_start_transpose(
                    hT[:, k, :], h_dram[:, k * 128:(k + 1) * 128])
            for t in range(NWIN):
                ps = n2p.tile([128, 128], f32, tag="ps2")
                nc.tensor.matmul(ps[:], T["ones16"][:], T["b2"][:],
                                 start=True, stop=False,
                                 skip_group_check=True)
                for k in range(2):
                    nc.tensor.matmul(
                        ps[:], hT[:, k, t * 128:(t + 1) * 128],
                        T["W2"][:, k * 128:(k + 1) * 128],
                        start=False, stop=(k == 1), skip_group_check=True)
                v = n2.tile([128, 128], f16, tag="v2")
                nc.vector.memset(v[:], 0.0)
                nc.vector.tensor_copy(v[:, 0:64], ps[:, 0:64])
                nc.sync.dma_start(l2_loc[t * 128:(t + 1) * 128, :], v[:])
                v2 = n2.tile([128, 128], f16, tag="v3")
                nc.vector.memset(v2[:], 0.0)
                nc.vector.tensor_copy(v2[:, 0:64], ps[:, 64:128])
                nc.sync.dma_start(r2_loc[t * 128:(t + 1) * 128, :], v2[:])

    nc.gpsimd.collective_compute(
        "AllGather", A.bypass, replica_groups=[list(range(NC))],
        ins=[l2_loc[:].opt()], outs=[l2_full[:].opt()])

    # ---- edge phase 2 -> output rows ----
    with (
        tc.tile_pool(name="e2", bufs=2) as e2,
        tc.tile_pool(name="e2s", bufs=1) as e2s,
        tc.tile_pool(name="e2p", bufs=4, space="PSUM") as e2p,
    ):
        _edge_layer(nc, tc, mybir, groups, T, e2, e2s, e2p,
                    layer=2, src_tab=l2_full, xr_tab=r2_loc, h=None)


def _edge_layer(nc, tc, mybir, groups, T, pool, spool, ppool, layer,
                src_tab, xr_tab, h):
    f16, f32 = mybir.dt.float16, mybir.dt.float32
    A = mybir.AluOpType
    AF = mybir.ActivationFunctionType
    CH = 256 if layer == 1 else 128   # gathered row width (f16 elems)
    CV = 256 if layer == 1 else 64    # valid channels
    NH = 4 if layer == 1 else 1       # heads
    CPH = CV // NH                    # channels per head
    RW = NH + CV                      # [w | wfeat]
    tg = f"l{layer}"

    for gi, g in enumerate(groups):
        nt = len(g["slots"])
        idxs = pool.tile([128, g["cols"]], mybir.dt.int16, tag=tg + "ix")
        nc.sync.dma_start(idxs[:], T["idx_d"][gi][:, :])
        xlg = pool.tile([128, nt, CH], f16, tag=tg + "xl")
        xrg = pool.tile([128, nt, CH], f16, tag=tg + "xr")
        co = 0
        for kind, s0, ntc in g["calls"]:
            if kind == "A":
                dst, src = xlg, src_tab[0:SPLIT, :]
            elif kind == "B":
                dst, src = xlg, src_tab[SPLIT:NROWS, :]
            else:
                dst, src = xrg, xr_tab[:, :]
            if STREAM_MODE:
                r0 = (g["ti"] * 64) % 4096
                nc.sync.dma_start(
                    dst[:, s0:s0 + ntc, :],
                    src[r0:r0 + ntc * 128, :].rearrange(
                        "(t p) c -> p t c", p=128))
            else:
                nc.gpsimd.dma_gather(
                    dst[:, s0:s0 + ntc, :], src, idxs[:, co:co + ntc * 8],
                    ntc * 128, ntc * 128, CH, single_packet=False)
            co += ntc * 8

        if layer == 1:
            xl3, xr3 = xlg[:], xrg[:]
        else:
            xl3, xr3 = xlg[:, :, 0:64], xrg[:, :, 0:64]
        m = spool.tile([128, nt, CV], f16, tag=tg + "m")
        nc.vector.tensor_tensor(m[:], xl3, xr3, A.add)
        t_ = spool.tile([128, nt, CV], f16, tag=tg + "t")
        nc.vector.scalar_tensor_tensor(t_[:], m[:], SLOPE, m[:],
                                       A.mult, A.max)
        sgn = (T["sg1"] if layer == 1 else T["sg2"])[:].unsqueeze(1)\
            .broadcast_to([128, nt, CV])
        nc.vector.tensor_tensor(m[:], t_[:], sgn, A.mult)
        e = spool.tile([128, nt, NH], f32, tag=tg + "e")
        nc.vector.reduce_sum(
            e[:], m[:].rearrange("p t (h c) -> p t h c", h=NH),
            axis=mybir.AxisListType.X)
        rhs = pool.tile([128, nt, RW], f16, tag=tg + "rhs")
        nc.scalar.activation(rhs[:, :, 0:NH], e[:], AF.Exp)
        wb = rhs[:, :, 0:NH].unsqueeze(3).broadcast_to([128, nt, NH, CPH])
        xl4 = xl3.rearrange("p t (h c) -> p t h c", h=NH)
        nc.vector.tensor_tensor(
            rhs[:, :, NH:RW].rearrange("p t (h c) -> p t h c", h=NH),
            xl4, wb, A.mult)

        ind = pool.tile([128, nt, 128], f16, tag=tg + "ind")
        iot_b = T["iot"][:].unsqueeze(1).broadcast_to([128, nt, 128])
        seg_b = T["seg"][:, g["ti"]:g["ti"] + nt].unsqueeze(2)\
            .broadcast_to([128, nt, 128])
        nc.vector.tensor_tensor(ind[:], iot_b, seg_b, A.is_equal)

        pstiles = {}
        last_slot = {}
        for si, (w, kind, k) in enumerate(g["slots"]):
            last_slot[w] = si
        for si, (w, kind, k) in enumerate(g["slots"]):
            st = w not in pstiles
            if st:
                pstiles[w] = ppool.tile([128, RW], f32, tag=tg + "ps",
                                        name=f"{tg}ps{w}")
            nc.tensor.matmul(pstiles[w][:], ind[:, si, :], rhs[:, si, :],
                             start=st, stop=(si == last_slot[w]),
                             skip_group_check=True)

        for w in g["ws"]:
            ps = pstiles[w]
            if layer == 1:
                rec = spool.tile([128, 4], f32, tag=tg + "rec")
                nc.vector.reciprocal(rec[:], ps[:, 0:4])
                u = spool.tile([128, 256], f16, tag=tg + "u")
                nc.vector.tensor_tensor(
                    u[:].rearrange("p (h c) -> p h c", h=4),
                    ps[:, 4:260].rearrange("p (h c) -> p h c", h=4),
                    rec[:].unsqueeze(2).broadcast_to([128, 4, 64]), A.mult)
                nc.vector.tensor_tensor(u[:], u[:], T["rc1"][:], A.mult)
                nc.vector.tensor_tensor(u[:], u[:], T["ba1"][:], A.add)
                lo = spool.tile([128, 256], f16, tag=tg + "lo")
                nc.vector.tensor_scalar_min(lo[:], u[:], 0.0)
                ex = spool.tile([128, 256], f16, tag=tg + "ex")
                nc.scalar.activation(ex[:], lo[:], AF.Exp)
                nc.vector.tensor_scalar_max(u[:], u[:], 0.0)
                hrow = pool.tile([128, 256], f16, tag=tg + "hrow")
                nc.vector.scalar_tensor_tensor(
                    hrow[:], ex[:], -1.0, u[:], A.add, A.add)
                nc.sync.dma_start(h[w * 128:(w + 1) * 128, :], hrow[:])
            else:
                rec = spool.tile([128, 1], f32, tag=tg + "rec2")
                nc.vector.reciprocal(rec[:], ps[:, 0:1])
                u = spool.tile([128, 64], f32, tag=tg + "u2")
                nc.vector.scalar_tensor_tensor(
                    u[:], ps[:, 1:65], rec[:], T["rc2"][:], A.mult, A.mult)
                nc.vector.tensor_tensor(u[:], u[:], T["ba2"][:], A.add)
                nc.sync.dma_start(T["out_d"][w * 128:(w + 1) * 128, :], u[:])


def _host_prep(inputs):
    att1 = np.asarray(inputs["att1"], np.float64)
    att2 = np.asarray(inputs["att2"], np.float64)[0]
    f1 = np.maximum(np.abs(att1.reshape(-1)), 1e-30)
    s1 = np.where(att1.reshape(-1) >= 0, 1.0, -1.0)
    f2 = np.maximum(np.abs(att2), 1e-30)
    s2 = np.where(att2 >= 0, 1.0, -1.0)

    W1 = np.concatenate([np.asarray(inputs["Wl1"], np.float64) * f1,
                         np.asarray(inputs["Wr1"], np.float64) * f1],
                        1)
    b1 = np.concatenate([np.asarray(inputs["bl1"], np.float64) * f1,
                         np.asarray(inputs["br1"], np.float64) * f1])
    W2c = np.concatenate([np.asarray(inputs["Wl2"], np.float64) * f2,
                          np.asarray(inputs["Wr2"], np.float64) * f2],
                         1)                      # [256, 128]
    W2 = np.concatenate([W2c[0:128], W2c[128:256]], 1)  # [128, 256] 2 chunks
    b2 = np.concatenate([np.asarray(inputs["bl2"], np.float64) * f2,
                         np.asarray(inputs["br2"], np.float64) * f2])

    com = dict(
        W1=W1.astype(np.float32), b1=b1.reshape(1, 512).astype(np.float32),
        W2=W2.astype(np.float16), b2=b2.reshape(1, 128).astype(np.float16),
        sg1=np.tile(s1.astype(np.float16), (128, 1)),
        sg2=np.tile(s2.astype(np.float16), (128, 1)),
        rc1=np.tile((1.0 / f1).astype(np.float16), (128, 1)),
        rc2=np.tile((1.0 / f2).astype(np.float32), (128, 1)),
        ba1=np.tile(np.asarray(inputs["bias1"], np.float16), (128, 1)),
        ba2=np.tile(np.asarray(inputs["bias2"], np.float32), (128, 1)),
        iot=np.tile(np.arange(128, dtype=np.int16), (128, 1)),
    )
    x = np.asarray(inputs["x"], np.float32)
    xTs = []
    for c in range(NC):
        xt = np.zeros((128, NPAD), np.float32)
        xt[:, 0:NLOC] = x[c * NLOC:(c + 1) * NLOC].T
        xTs.append(xt)
    return com, xTs


def _get_built(edge_index, rep=1):
    key = (hash(np.asarray(edge_index).tobytes()), rep)
    if key not in _cache:
        groups, seg_all, idx_all, tot_tiles = _host_metadata(edge_index)
        nc = _build(groups, tot_tiles, rep=rep)
        _cache[key] = (groups, seg_all, idx_all, nc)
    return _cache[key]


def make_maps(inputs, seg_all, idx_all):
    com, xTs = _host_prep(inputs)
    maps = []
    for c in range(NC):
        m = dict(com)
        m["xT"] = xTs[c]
        m["seg"] = seg_all[c]
        for gi in range(len(idx_all)):
            m[f"idx{gi}"] = idx_all[gi][c]
        maps.append(m)
    return maps


def kernel(**inputs):
    from concourse.bass_utils import run_bass_kernel_spmd

    groups, seg_all, idx_all, nc = _get_built(inputs["edge_index"])
    maps = make_maps(inputs, seg_all, idx_all)
    res = run_bass_kernel_spmd(nc, maps, list(range(NC)))
    out = np.zeros((N, OUT), np.float32)
    for c in range(NC):
        out[c * NLOC:(c + 1) * NLOC] = res.results[c]["out"][0:NLOC]
    return out



# revision 16
# speedup vs baseline: 149.9306x; 149.9306x over previous
"""GATv2 2-layer GNN on 8 Trainium2 NeuronCores (Bass/Tile).

Sharding: dst-range (6250 dsts/core), edges dst-sorted into 49 windows of
128 dsts. Per-edge endpoint rows are fetched with dma_gather from f16 tables
(512B rows, int16 indices -> tables split at row 25088 into A/B halves).
The |att|-fold plus a +-1 sign mask turns sum_c att_c*lrelu(m_c) into
lrelu + masked reduce. Segment softmax skips max-subtraction (e stays in
[-7, 7] for this model; exp in f16 is safe). Segment sums run on the PE as
0/1-indicator matmuls accumulated into one PSUM tile per window. Layer 2
reuses the same index streams on its own tables. The program is
SPMD-uniform: per-window tile counts are maxed over cores; cores pad with
idx=0 / seg=128 slots which contribute zero.
"""
import sys

sys.path.insert(0, "/opt/trn_rl_repo")

import numpy as np

N = 50000
IN, HID, H, OUT = 128, 64, 4, 64
SLOPE = 0.2
NC = 8
NLOC = N // NC            # 6250
NPAD = 6272               # 49*128
NWIN = NPAD // 128        # 49
NROWS = NPAD * NC         # 50176
SPLIT = NROWS // 2        # 25088
GW = 2                    # windows per gather group
STREAM_MODE = False       # debug: replace gathers with sequential streams
CALL_CAP = 16             # max tiles (x128 idx) per dma_gather call
MODE = "full"             # full | stream | nogather | gather_only | noedge

_cache = {}


def _wrap16(stream):
    n = len(stream)
    a = np.zeros((16, n // 16), np.int16)
    a[np.arange(n) % 16, np.arange(n) // 16] = stream
    return np.tile(a, (8, 1))


def _host_metadata(edge_index):
    src = np.concatenate([np.asarray(edge_index[0], np.int64), np.arange(N)])
    dst = np.concatenate([np.asarray(edge_index[1], np.int64), np.arange(N)])
    srow = (src // NLOC) * NPAD + (src % NLOC)
    dcore = dst // NLOC
    dloc = dst % NLOC

    per_cw = [[None] * NWIN for _ in range(NC)]
    for c in range(NC):
        m = dcore == c
        sr, dl = srow[m], dloc[m]
        o = np.argsort(dl, kind="stable")
        sr, dl = sr[o], dl[o]
        wi = dl // 128
        for w in range(NWIN):
            ws = wi == w
            s_w, d_w = sr[ws], dl[ws] - w * 128
            a = s_w < SPLIT
            per_cw[c][w] = (s_w[a], d_w[a], s_w[~a] - SPLIT, d_w[~a])

    TA = [max((len(per_cw[c][w][0]) + 127) // 128 for c in range(NC))
          for w in range(NWIN)]
    TB = [max((len(per_cw[c][w][2]) + 127) // 128 for c in range(NC))
          for w in range(NWIN)]

    groups = []
    w = 0
    ti = 0
    while w < NWIN:
        ws = list(range(w, min(w + GW, NWIN)))
        na = sum(TA[x] for x in ws)
        nb = sum(TB[x] for x in ws)
        slots = [(x, "A", i) for x in ws for i in range(TA[x])] + \
                [(x, "B", i) for x in ws for i in range(TB[x])]
        t = len(slots)
        calls = []
        for kind, lo, hi in (("A", 0, na), ("B", na, na + nb)):
            s0 = lo
            while s0 < hi:
                n = min(CALL_CAP, hi - s0)
                calls.append((kind, s0, n))
                s0 += n
        groups.append(dict(ws=ws, na=na, nb=nb, slots=slots, calls=calls,
                           cols=sum(n * 8 for _, _, n in calls), ti=ti))
        ti += t
        w += GW
    tot_tiles = ti

    seg_all = np.full((NC, 128, tot_tiles), 128, np.int16)
    idx_all = [np.zeros((NC, 128, g["cols"]), np.int16) for g in groups]
    for gi, g in enumerate(groups):
        nslot = len(g["slots"])
        for c in range(NC):
            slot_idx = np.zeros((nslot, 128), np.int64)
            for si, (w, kind, k) in enumerate(g["slots"]):
                sa, da, sb, db = per_cw[c][w]
                s_, d_ = (sa, da) if kind == "A" else (sb, db)
                iv = s_[k * 128:(k + 1) * 128]
                dv = d_[k * 128:(k + 1) * 128]
                n = len(iv)
                slot_idx[si, :n] = iv
                seg_all[c, :n, g["ti"] + si] = dv
            co = 0
            for kind, s0, ntc in g["calls"]:
                idx_all[gi][c, :, co:co + ntc * 8] = _wrap16(
                    slot_idx[s0:s0 + ntc].reshape(-1))
                co += ntc * 8
    return groups, seg_all, idx_all, tot_tiles


def _build(groups, tot_tiles, rep=1):
    import concourse.bacc as bacc
    import concourse.mybir as mybir
    import concourse.tile as tile
    from concourse import library_config

    f16, f32, i16 = mybir.dt.float16, mybir.dt.float32, mybir.dt.int16
    A = mybir.AluOpType

    nc = bacc.Bacc("TRN2", target_bir_lowering=False, debug=False,
                   num_devices=NC)
    d = {}
    for name, shape, dt in [
            ("xT", [128, NPAD], f32), ("W1", [128, 512], f32),
            ("b1", [1, 512], f32), ("W2", [128, 256], f16),
            ("b2", [1, 128], f16), ("sg1", [128, 256], f16),
            ("sg2", [128, 64], f16), ("rc1", [128, 256], f16),
            ("rc2", [128, 64], f32), ("ba1", [128, 256], f16),
            ("ba2", [128, 64], f32), ("iot", [128, 128], i16),
            ("ident", [128, 128], f16),
            ("seg", [128, tot_tiles], i16)]:
        d[name] = nc.dram_tensor(name, shape, dt, kind="ExternalInput")
    idx_d = [nc.dram_tensor(f"idx{gi}", [128, g["cols"]], i16,
                            kind="ExternalInput")
             for gi, g in enumerate(groups)]
    out_d = nc.dram_tensor("out", [NPAD, 64], f32, kind="ExternalOutput")

    with tile.TileContext(nc) as tc:
        with (
            tc.tile_pool(name="const", bufs=1) as cp,
            tc.tile_pool(name="dram", bufs=1, space="DRAM") as dp,
        ):
            nc.gpsimd.load_library(library_config.mlp)
            T = {"out_d": out_d, "dp": dp, "idx_d": idx_d}
            for name in d:
                tl = cp.tile(list(d[name].shape), d[name].dtype, tag=name)
                nc.sync.dma_start(tl[:], d[name][:, :])
                T[name] = tl
            ones = cp.tile([1, 128], f32)
            nc.vector.memset(ones[:], 1.0)
            ones16 = cp.tile([1, 128], f16)
            nc.vector.memset(ones16[:], 1.0)
            T["ones"], T["ones16"] = ones, ones16

            for r in range(rep):
                _one_pass(nc, tc, mybir, groups, T)
    nc.compile()
    return nc


def _one_pass(nc, tc, mybir, groups, T):
    f16, f32 = mybir.dt.float16, mybir.dt.float32
    A = mybir.AluOpType
    dp = T["dp"]

    xl_loc = dp.tile([NPAD, 256], f16, tag="xl_loc")
    xl_full = dp.tile([NROWS, 256], f16, tag="xl_full")
    l2_loc = dp.tile([NPAD, 128], f16, tag="l2_loc")
    l2_full = dp.tile([NROWS, 128], f16, tag="l2_full")

    with tc.tile_pool(name="acc", bufs=1) as accp:
        # xr rows for local windows stay SBUF-resident (edge phase reads
        # window w's 128 dst rows as a [128, 256] slice).
        xracc = accp.tile([128, NWIN, 256], f16)
        r2acc = accp.tile([128, NWIN, 64], f16)

        # ---- node phase 1: att-folded xl/xr tables for local nodes ----
        with (
            tc.tile_pool(name="n1", bufs=3) as n1,
            tc.tile_pool(name="n1p", bufs=2, space="PSUM") as n1p,
        ):
            for t in range(NWIN):
                ps = n1p.tile([128, 512], f32, tag="ps")
                nc.tensor.matmul(ps[:], T["ones"][:], T["b1"][:],
                                 start=True, stop=False, skip_group_check=True)
                nc.tensor.matmul(ps[:], T["xT"][:, t * 128:(t + 1) * 128],
                                 T["W1"][:], start=False, stop=True,
                                 skip_group_check=True)
                v = n1.tile([128, 256], f16, tag="v")
                nc.scalar.copy(v[:], ps[:, 0:256])
                nc.scalar.copy(xracc[:, t, :], ps[:, 256:512])
                nc.sync.dma_start(xl_loc[t * 128:(t + 1) * 128, :], v[:])

        nc.gpsimd.collective_compute(
            "AllGather", A.bypass, replica_groups=[list(range(NC))],
            ins=[xl_loc[:].opt()], outs=[xl_full[:].opt()])

        # ---- edge phase 1 -> h rows in DRAM ----
        h_dram = dp.tile([NPAD, 256], f16, tag="h_dram")
        if MODE != "noedge":
            with (
                tc.tile_pool(name="e1", bufs=2) as e1,
                tc.tile_pool(name="e1s", bufs=1) as e1s,
                tc.tile_pool(name="e1p", bufs=2, space="PSUM") as e1p,
                tc.tile_pool(name="e1px", bufs=3, space="PSUM") as e1px,
            ):
                _edge_layer(nc, tc, mybir, groups, T, e1, e1s, e1p, e1px,
                            layer=1, src_tab=xl_full, xr_acc=xracc, h=h_dram)

            # ---- node phase 2: hl2/hr2 tables ----
            if MODE == "gather_only":
                return
            with (
                tc.tile_pool(name="hT", bufs=1) as hTp,
                tc.tile_pool(name="n2", bufs=3) as n2,
                tc.tile_pool(name="n2p", bufs=2, space="PSUM") as n2p,
            ):
                hT = hTp.tile([128, 2, NPAD], f16)
                for k in range(2):
                    nc.sync.dma_start_transpose(
                        hT[:, k, :], h_dram[:, k * 128:(k + 1) * 128])
                for t in range(NWIN):
                    ps = n2p.tile([128, 128], f32, tag="ps2")
                    nc.tensor.matmul(ps[:], T["ones16"][:], T["b2"][:],
                                     start=True, stop=False,
                                     skip_group_check=True)
                    for k in range(2):
                        nc.tensor.matmul(
                            ps[:], hT[:, k, t * 128:(t + 1) * 128],
                            T["W2"][:, k * 128:(k + 1) * 128],
                            start=False, stop=(k == 1), skip_group_check=True)
                    v = n2.tile([128, 128], f16, tag="v2")
                    nc.vector.memset(v[:, 64:128], 0.0)
                    nc.vector.tensor_copy(v[:, 0:64], ps[:, 0:64])
                    nc.sync.dma_start(l2_loc[t * 128:(t + 1) * 128, :], v[:])
                    nc.scalar.copy(r2acc[:, t, :], ps[:, 64:128])

        nc.gpsimd.collective_compute(
            "AllGather", A.bypass, replica_groups=[list(range(NC))],
            ins=[l2_loc[:].opt()], outs=[l2_full[:].opt()])

        # ---- edge phase 2 -> output rows ----
        if MODE not in ("noedge", "gather_only"):
            with (
                tc.tile_pool(name="e2", bufs=2) as e2,
                tc.tile_pool(name="e2s", bufs=1) as e2s,
                tc.tile_pool(name="e2p", bufs=2, space="PSUM") as e2p,
                tc.tile_pool(name="e2px", bufs=3, space="PSUM") as e2px,
            ):
                _edge_layer(nc, tc, mybir, groups, T, e2, e2s, e2p, e2px,
                            layer=2, src_tab=l2_full, xr_acc=r2acc, h=None)


def _edge_layer(nc, tc, mybir, groups, T, pool, spool, ppool, xpool, layer,
                src_tab, xr_acc, h):
    f16, f32 = mybir.dt.float16, mybir.dt.float32
    A = mybir.AluOpType
    AF = mybir.ActivationFunctionType
    CH = 256 if layer == 1 else 128   # gathered row width (f16 elems)
    CV = 256 if layer == 1 else 64    # valid channels
    NH = 4 if layer == 1 else 1       # heads
    CPH = CV // NH                    # channels per head
    RW = NH + CV                      # [w | wfeat]
    tg = f"l{layer}"

    for gi, g in enumerate(groups):
        nt = len(g["slots"])
        idxs = pool.tile([128, g["cols"]], mybir.dt.int16, tag=tg + "ix")
        nc.sync.dma_start(idxs[:], T["idx_d"][gi][:, :])
        xlg = pool.tile([128, nt, CH], f16, tag=tg + "xl")
        co = 0
        for kind, s0, ntc in g["calls"]:
            src = src_tab[0:SPLIT, :] if kind == "A" else src_tab[SPLIT:NROWS, :]
            if MODE == "nogather":
                nc.vector.memset(xlg[:, s0:s0 + ntc, 0:8], 0.0)
            elif STREAM_MODE or MODE == "stream":
                r0 = (g["ti"] * 64) % 4096
                nc.sync.dma_start(
                    xlg[:, s0:s0 + ntc, :],
                    src[r0:r0 + ntc * 128, :].rearrange(
                        "(t p) c -> p t c", p=128))
            else:
                nc.gpsimd.dma_gather(
                    xlg[:, s0:s0 + ntc, :], src, idxs[:, co:co + ntc * 8],
                    ntc * 128, ntc * 128, CH, single_packet=False)
            co += ntc * 8

        ind = pool.tile([128, nt, 128], f16, tag=tg + "ind")
        iot_b = T["iot"][:].unsqueeze(1).broadcast_to([128, nt, 128])
        seg_b = T["seg"][:, g["ti"]:g["ti"] + nt].unsqueeze(2)\
            .broadcast_to([128, nt, 128])
        nc.vector.tensor_tensor(ind[:], iot_b, seg_b, A.is_equal)

        # per-edge xr rows via transposed-indicator matmuls (no DMA gather)
        indT = pool.tile([128, nt, 128], f16, tag=tg + "indT")
        xr_sb = pool.tile([128, nt, CV], f16, tag=tg + "xrsb")
        for si, (w, kind, k) in enumerate(g["slots"]):
            psT = xpool.tile([128, 128], f16, tag=tg + "psT",
                             name=f"{tg}psT")
            nc.tensor.transpose(psT[:], ind[:, si, :], T["ident"][:])
            nc.scalar.copy(indT[:, si, :], psT[:])
            xre = xpool.tile([128, CV], f32, tag=tg + "xre",
                             name=f"{tg}xre")
            nc.tensor.matmul(xre[:], indT[:, si, :], xr_acc[:, w, 0:CV],
                             start=True, stop=True, skip_group_check=True)
            nc.scalar.copy(xr_sb[:, si, :], xre[:])

        if MODE == "gather_only":
            return

        xl3 = xlg[:] if layer == 1 else xlg[:, :, 0:64]
        m = spool.tile([128, nt, CV], f16, tag=tg + "m")
        nc.vector.tensor_tensor(m[:], xl3, xr_sb[:], A.add)
        t_ = spool.tile([128, nt, CV], f16, tag=tg + "t")
        nc.vector.scalar_tensor_tensor(t_[:], m[:], SLOPE, m[:],
                                       A.mult, A.max)
        sgn = (T["sg1"] if layer == 1 else T["sg2"])[:].unsqueeze(1)\
            .broadcast_to([128, nt, CV])
        nc.vector.tensor_tensor(m[:], t_[:], sgn, A.mult)
        e = spool.tile([128, nt, NH], f32, tag=tg + "e")
        nc.vector.reduce_sum(
            e[:], m[:].rearrange("p t (h c) -> p t h c", h=NH),
            axis=mybir.AxisListType.X)
        rhs = pool.tile([128, nt, RW], f16, tag=tg + "rhs")
        nc.scalar.activation(rhs[:, :, 0:NH], e[:], AF.Exp)
        wb = rhs[:, :, 0:NH].unsqueeze(3).broadcast_to([128, nt, NH, CPH])
        xl4 = xl3.rearrange("p t (h c) -> p t h c", h=NH)
        nc.vector.tensor_tensor(
            rhs[:, :, NH:RW].rearrange("p t (h c) -> p t h c", h=NH),
            xl4, wb, A.mult)

        pstiles = {}
        last_slot = {}
        for si, (w, kind, k) in enumerate(g["slots"]):
            last_slot[w] = si
        for si, (w, kind, k) in enumerate(g["slots"]):
            st = w not in pstiles
            if st:
                pstiles[w] = ppool.tile([128, RW], f32, tag=tg + "ps",
                                        name=f"{tg}ps{w}")
            nc.tensor.matmul(pstiles[w][:], ind[:, si, :], rhs[:, si, :],
                             start=st, stop=(si == last_slot[w]),
                             skip_group_check=True)

        for w in g["ws"]:
            ps = pstiles[w]
            if layer == 1:
                rec = spool.tile([128, 4], f32, tag=tg + "rec")
                nc.vector.reciprocal(rec[:], ps[:, 0:4])
                u = spool.tile([128, 256], f16, tag=tg + "u")
                nc.vector.tensor_tensor(
                    u[:].rearrange("p (h c) -> p h c", h=4),
                    ps[:, 4:260].rearrange("p (h c) -> p h c", h=4),
                    rec[:].unsqueeze(2).broadcast_to([128, 4, 64]), A.mult)
                nc.vector.tensor_tensor(u[:], u[:], T["rc1"][:], A.mult)
                nc.vector.tensor_tensor(u[:], u[:], T["ba1"][:], A.add)
                lo = spool.tile([128, 256], f16, tag=tg + "lo")
                nc.vector.tensor_scalar_min(lo[:], u[:], 0.0)
                ex = spool.tile([128, 256], f16, tag=tg + "ex")
                nc.scalar.activation(ex[:], lo[:], AF.Exp)
                nc.vector.tensor_scalar_max(u[:], u[:], 0.0)
                hrow = pool.tile([128, 256], f16, tag=tg + "hrow")
                nc.vector.scalar_tensor_tensor(
                    hrow[:], ex[:], -1.0, u[:], A.add, A.add)
                nc.sync.dma_start(h[w * 128:(w + 1) * 128, :], hrow[:])
            else:
                rec = spool.tile([128, 1], f32, tag=tg + "rec2")
                nc.vector.reciprocal(rec[:], ps[:, 0:1])
                u = spool.tile([128, 64], f32, tag=tg + "u2")
                nc.vector.scalar_tensor_tensor(
                    u[:], ps[:, 1:65], rec[:], T["rc2"][:], A.mult, A.mult)
                nc.vector.tensor_tensor(u[:], u[:], T["ba2"][:], A.add)
                nc.sync.dma_start(T["out_d"][w * 128:(w + 1) * 128, :], u[:])


def _host_prep(inputs):
    att1 = np.asarray(inputs["att1"], np.float64)
    att2 = np.asarray(inputs["att2"], np.float64)[0]
    f1 = np.maximum(np.abs(att1.reshape(-1)), 1e-30)
    s1 = np.where(att1.reshape(-1) >= 0, 1.0, -1.0)
    f2 = np.maximum(np.abs(att2), 1e-30)
    s2 = np.where(att2 >= 0, 1.0, -1.0)

    W1 = np.concatenate([np.asarray(inputs["Wl1"], np.float64) * f1,
                         np.asarray(inputs["Wr1"], np.float64) * f1],
                        1)
    b1 = np.concatenate([np.asarray(inputs["bl1"], np.float64) * f1,
                         np.asarray(inputs["br1"], np.float64) * f1])
    W2c = np.concatenate([np.asarray(inputs["Wl2"], np.float64) * f2,
                          np.asarray(inputs["Wr2"], np.float64) * f2],
                         1)                      # [256, 128]
    W2 = np.concatenate([W2c[0:128], W2c[128:256]], 1)  # [128, 256] 2 chunks
    b2 = np.concatenate([np.asarray(inputs["bl2"], np.float64) * f2,
                         np.asarray(inputs["br2"], np.float64) * f2])

    com = dict(
        W1=W1.astype(np.float32), b1=b1.reshape(1, 512).astype(np.float32),
        W2=W2.astype(np.float16), b2=b2.reshape(1, 128).astype(np.float16),
        sg1=np.tile(s1.astype(np.float16), (128, 1)),
        sg2=np.tile(s2.astype(np.float16), (128, 1)),
        rc1=np.tile((1.0 / f1).astype(np.float16), (128, 1)),
        rc2=np.tile((1.0 / f2).astype(np.float32), (128, 1)),
        ba1=np.tile(np.asarray(inputs["bias1"], np.float16), (128, 1)),
        ba2=np.tile(np.asarray(inputs["bias2"], np.float32), (128, 1)),
        iot=np.tile(np.arange(128, dtype=np.int16), (128, 1)),
        ident=np.eye(128, dtype=np.float16),
    )
    x = np.asarray(inputs["x"], np.float32)
    xTs = []
    for c in range(NC):
        xt = np.zeros((128, NPAD), np.float32)
        xt[:, 0:NLOC] = x[c * NLOC:(c + 1) * NLOC].T
        xTs.append(xt)
    return com, xTs


def _get_built(edge_index, rep=1):
    key = (hash(np.asarray(edge_index).tobytes()), rep, MODE)
    if key not in _cache:
        groups, seg_all, idx_all, tot_tiles = _host_metadata(edge_index)
        nc = _build(groups, tot_tiles, rep=rep)
        _cache[key] = (groups, seg_all, idx_all, nc)
    return _cache[key]


def make_maps(inputs, seg_all, idx_all):
    com, xTs = _host_prep(inputs)
    maps = []
    for c in range(NC):
        m = dict(com)
        m["xT"] = xTs[c]
        m["seg"] = seg_all[c]
        for gi in range(len(idx_all)):
            m[f"idx{gi}"] = idx_all[gi][c]
        maps.append(m)
    return maps


def kernel(**inputs):
    from concourse.bass_utils import run_bass_kernel_spmd

    groups, seg_all, idx_all, nc = _get_built(inputs["edge_index"])
    maps = make_maps(inputs, seg_all, idx_all)
    res = run_bass_kernel_spmd(nc, maps, list(range(NC)))
    out = np.zeros((N, OUT), np.float32)
    for c in range(NC):
        out[c * NLOC:(c + 1) * NLOC] = res.results[c]["out"][0:NLOC]
    return out



# revision 28
# speedup vs baseline: 169.2080x; 1.1286x over previous
"""GATv2 2-layer GNN on 8 Trainium2 NeuronCores (Bass/Tile).

Sharding: dst-range (6250 dsts/core), edges dst-sorted into 49 windows of
128 dsts. Per-edge endpoint rows are fetched with dma_gather from f16 tables
(512B rows, int16 indices -> tables split at row 25088 into A/B halves).
The |att|-fold plus a +-1 sign mask turns sum_c att_c*lrelu(m_c) into
lrelu + masked reduce. Segment softmax skips max-subtraction (e stays in
[-7, 7] for this model; exp in f16 is safe). Segment sums run on the PE as
0/1-indicator matmuls accumulated into one PSUM tile per window. Layer 2
reuses the same index streams on its own tables. The program is
SPMD-uniform: per-window tile counts are maxed over cores; cores pad with
idx=0 / seg=128 slots which contribute zero.
"""
import sys

sys.path.insert(0, "/opt/trn_rl_repo")

import numpy as np

N = 50000
IN, HID, H, OUT = 128, 64, 4, 64
SLOPE = 0.2
NC = 8
NLOC = N // NC            # 6250
NPAD = 6272               # 49*128
NWIN = NPAD // 128        # 49
NROWS = NPAD * NC         # 50176
SPLIT = NROWS // 2        # 25088
GW = 2                    # windows per gather group
STREAM_MODE = False       # debug: replace gathers with sequential streams
CALL_CAP = 16             # max tiles (x128 idx) per dma_gather call
MODE = "full"             # full | stream | nogather | gather_only | noedge

_cache = {}


def _wrap16(stream):
    n = len(stream)
    a = np.zeros((16, n // 16), np.int16)
    a[np.arange(n) % 16, np.arange(n) // 16] = stream
    return np.tile(a, (8, 1))


def _host_metadata(edge_index):
    src = np.concatenate([np.asarray(edge_index[0], np.int64), np.arange(N)])
    dst = np.concatenate([np.asarray(edge_index[1], np.int64), np.arange(N)])
    srow = (src // NLOC) * NPAD + (src % NLOC)
    dcore = dst // NLOC
    dloc = dst % NLOC

    per_cw = [[None] * NWIN for _ in range(NC)]
    for c in range(NC):
        m = dcore == c
        sr, dl = srow[m], dloc[m]
        o = np.argsort(dl, kind="stable")
        sr, dl = sr[o], dl[o]
        wi = dl // 128
        for w in range(NWIN):
            ws = wi == w
            s_w, d_w = sr[ws], dl[ws] - w * 128
            a = s_w < SPLIT
            per_cw[c][w] = (s_w[a], d_w[a], s_w[~a] - SPLIT, d_w[~a])

    TA = [max((len(per_cw[c][w][0]) + 127) // 128 for c in range(NC))
          for w in range(NWIN)]
    TB = [max((len(per_cw[c][w][2]) + 127) // 128 for c in range(NC))
          for w in range(NWIN)]

    groups = []
    w = 0
    ti = 0
    while w < NWIN:
        ws = list(range(w, min(w + GW, NWIN)))
        na = sum(TA[x] for x in ws)
        nb = sum(TB[x] for x in ws)
        slots = [(x, "A", i) for x in ws for i in range(TA[x])] + \
                [(x, "B", i) for x in ws for i in range(TB[x])]
        t = len(slots)
        calls = []
        for kind, lo, hi in (("A", 0, na), ("B", na, na + nb)):
            s0 = lo
            while s0 < hi:
                n = min(CALL_CAP, hi - s0)
                calls.append((kind, s0, n))
                s0 += n
        groups.append(dict(ws=ws, na=na, nb=nb, slots=slots, calls=calls,
                           cols=sum(n * 8 for _, _, n in calls), ti=ti))
        ti += t
        w += GW
    tot_tiles = ti
    co0 = 0
    for g in groups:
        g["co0"] = co0
        co0 += g["cols"]

    seg_all = np.full((NC, 128, tot_tiles), 128, np.int16)
    idx_all = [np.zeros((NC, 128, g["cols"]), np.int16) for g in groups]
    for gi, g in enumerate(groups):
        nslot = len(g["slots"])
        for c in range(NC):
            slot_idx = np.zeros((nslot, 128), np.int64)
            for si, (w, kind, k) in enumerate(g["slots"]):
                sa, da, sb, db = per_cw[c][w]
                s_, d_ = (sa, da) if kind == "A" else (sb, db)
                iv = s_[k * 128:(k + 1) * 128]
                dv = d_[k * 128:(k + 1) * 128]
                n = len(iv)
                slot_idx[si, :n] = iv
                seg_all[c, :n, g["ti"] + si] = dv
            co = 0
            for kind, s0, ntc in g["calls"]:
                idx_all[gi][c, :, co:co + ntc * 8] = _wrap16(
                    slot_idx[s0:s0 + ntc].reshape(-1))
                co += ntc * 8
    return groups, seg_all, idx_all, tot_tiles


def _build(groups, tot_tiles, rep=1):
    import concourse.bacc as bacc
    import concourse.mybir as mybir
    import concourse.tile as tile
    from concourse import library_config

    f16, f32, i16 = mybir.dt.float16, mybir.dt.float32, mybir.dt.int16
    A = mybir.AluOpType

    nc = bacc.Bacc("TRN2", target_bir_lowering=False, debug=False,
                   num_devices=NC, num_swdge_queues=2)
    totcols = sum(g["cols"] for g in groups)
    d = {}
    for name, shape, dt in [
            ("xT", [128, NPAD], f16), ("W1", [128, 512], f16),
            ("b1r", [128, 512], f16), ("W2", [128, 256], f16),
            ("b2r", [128, 128], f16), ("sg1", [128, 256], f16),
            ("sg2", [128, 64], f16), ("rc1", [128, 256], f16),
            ("rc2", [128, 64], f32), ("ba1", [128, 256], f16),
            ("ba2", [128, 64], f32), ("iot", [128, 128], i16),
            ("ident", [128, 128], f16),
            ("idxall", [128, totcols], i16),
            ("seg", [128, tot_tiles], i16)]:
        d[name] = nc.dram_tensor(name, shape, dt, kind="ExternalInput")
    out_d = nc.dram_tensor("out", [NPAD, 64], f32, kind="ExternalOutput")

    with tile.TileContext(nc) as tc:
        with (
            tc.tile_pool(name="const", bufs=1) as cp,
            tc.tile_pool(name="dram", bufs=1, space="DRAM") as dp,
        ):
            nc.gpsimd.load_library(library_config.mlp)
            T = {"out_d": out_d, "dp": dp, "xT_d": d["xT"]}
            for name in d:
                if name == "xT":
                    continue
                tl = cp.tile(list(d[name].shape), d[name].dtype, tag=name)
                nc.sync.dma_start(tl[:], d[name][:, :])
                T[name] = tl

            for r in range(rep):
                _one_pass(nc, tc, mybir, groups, T)
    nc.compile()
    return nc


def _one_pass(nc, tc, mybir, groups, T):
    f16, f32 = mybir.dt.float16, mybir.dt.float32
    A = mybir.AluOpType
    dp = T["dp"]

    xl_loc = dp.tile([NPAD, 256], f16, tag="xl_loc")
    xl_full = dp.tile([NROWS, 256], f16, tag="xl_full",
                      addr_space="Shared")
    l2_loc = dp.tile([NPAD, 128], f16, tag="l2_loc")
    l2_full = dp.tile([NROWS, 128], f16, tag="l2_full",
                      addr_space="Shared")

    with tc.tile_pool(name="acc", bufs=1) as accp:
        # whole-pass SBUF residents: xr/r2 window rows, transposed h, and
        # the gather index streams (shared by both edge layers).
        xracc = accp.tile([128, NWIN, 256], f16)
        r2acc = accp.tile([128, NWIN, 64], f16)
        hTacc = accp.tile([128, 2, NPAD], f16)
        idxs = accp.tile([128, T["idxall"].shape[1]], mybir.dt.int16)
        nc.sync.dma_start(idxs[:], T["idxall"][:])
        T["idxs"] = idxs

        # ---- node phase 1: att-folded xl/xr tables for local nodes ----
        with (
            tc.tile_pool(name="n1", bufs=1) as n1,
            tc.tile_pool(name="n1p", bufs=2, space="PSUM") as n1p,
        ):
            vlacc = n1.tile([128, NWIN, 256], f16)
            xTs = n1.tile([128, NPAD], f16)
            nc.sync.dma_start(xTs[:], T["xT_d"][:, :])
            for t in range(NWIN):
                ps = n1p.tile([128, 512], f32, tag="ps")
                nc.tensor.matmul(ps[:], xTs[:, t * 128:(t + 1) * 128],
                                 T["W1"][:], start=True, stop=True,
                                 skip_group_check=True)
                nc.vector.tensor_tensor(vlacc[:, t, :], ps[:, 0:256],
                                        T["b1r"][:, 0:256], A.add)
                nc.vector.tensor_tensor(xracc[:, t, :], ps[:, 256:512],
                                        T["b1r"][:, 256:512], A.add)
            nc.sync.dma_start(
                xl_loc[:, :].rearrange("(t p) c -> p t c", p=128), vlacc[:])

        nc.gpsimd.collective_compute(
            "AllGather", A.bypass, replica_groups=[list(range(NC))],
            ins=[xl_loc[:].opt()], outs=[xl_full[:].opt()])

        # ---- edge phase 1 -> hT rows in SBUF ----
        if MODE != "noedge":
            with (
                tc.tile_pool(name="e1", bufs=2) as e1,
                tc.tile_pool(name="e1s", bufs=1) as e1s,
                tc.tile_pool(name="e1p", bufs=2, space="PSUM") as e1p,
                tc.tile_pool(name="e1px", bufs=2, space="PSUM") as e1px,
            ):
                _edge_layer(nc, tc, mybir, groups, T, e1, e1s, e1p, e1px,
                            layer=1, src_tab=xl_full, xr_acc=xracc,
                            h=hTacc)

            # ---- node phase 2: hl2/hr2 tables ----
            if MODE == "gather_only":
                return
            with (
                tc.tile_pool(name="n2", bufs=1) as n2,
                tc.tile_pool(name="n2p", bufs=2, space="PSUM") as n2p,
            ):
                v2acc = n2.tile([128, NWIN, 128], f16)
                for t in range(NWIN):
                    ps = n2p.tile([128, 128], f32, tag="ps2")
                    for k in range(2):
                        nc.tensor.matmul(
                            ps[:], hTacc[:, k, t * 128:(t + 1) * 128],
                            T["W2"][:, k * 128:(k + 1) * 128],
                            start=(k == 0), stop=(k == 1),
                            skip_group_check=True)
                    nc.vector.memset(v2acc[:, t, 64:128], 0.0)
                    nc.vector.tensor_tensor(v2acc[:, t, 0:64], ps[:, 0:64],
                                            T["b2r"][:, 0:64], A.add)
                    nc.vector.tensor_tensor(r2acc[:, t, :], ps[:, 64:128],
                                            T["b2r"][:, 64:128], A.add)
                nc.sync.dma_start(
                    l2_loc[:, :].rearrange("(t p) c -> p t c", p=128),
                    v2acc[:])

        nc.gpsimd.collective_compute(
            "AllGather", A.bypass, replica_groups=[list(range(NC))],
            ins=[l2_loc[:].opt()], outs=[l2_full[:].opt()])

        # ---- edge phase 2 -> output rows ----
        if MODE not in ("noedge", "gather_only"):
            with (
                tc.tile_pool(name="e2", bufs=2) as e2,
                tc.tile_pool(name="e2s", bufs=1) as e2s,
                tc.tile_pool(name="e2p", bufs=2, space="PSUM") as e2p,
                tc.tile_pool(name="e2px", bufs=2, space="PSUM") as e2px,
            ):
                _edge_layer(nc, tc, mybir, groups, T, e2, e2s, e2p, e2px,
                            layer=2, src_tab=l2_full, xr_acc=r2acc, h=None)


def _edge_layer(nc, tc, mybir, groups, T, pool, spool, ppool, xpool, layer,
                src_tab, xr_acc, h):
    f16, f32 = mybir.dt.float16, mybir.dt.float32
    A = mybir.AluOpType
    AF = mybir.ActivationFunctionType
    CH = 256 if layer == 1 else 128   # gathered row width (f16 elems)
    CV = 256 if layer == 1 else 64    # valid channels
    NH = 4 if layer == 1 else 1       # heads
    CPH = CV // NH                    # channels per head
    RW = NH + CV                      # [w | wfeat]
    tg = f"l{layer}"

    for gi, g in enumerate(groups):
        nt = len(g["slots"])
        idxs = T["idxs"]
        xlg = pool.tile([128, nt, CH], f16, tag=tg + "xl")
        co = g["co0"]
        for ci, (kind, s0, ntc) in enumerate(g["calls"]):
            src = src_tab[0:SPLIT, :] if kind == "A" else src_tab[SPLIT:NROWS, :]
            if MODE == "nogather":
                nc.vector.memset(xlg[:, s0:s0 + ntc, 0:8], 0.0)
            elif STREAM_MODE or MODE == "stream":
                r0 = (g["ti"] * 64) % 4096
                nc.sync.dma_start(
                    xlg[:, s0:s0 + ntc, :],
                    src[r0:r0 + ntc * 128, :].rearrange(
                        "(t p) c -> p t c", p=128))
            else:
                nc.gpsimd.dma_gather(
                    xlg[:, s0:s0 + ntc, :], src, idxs[:, co:co + ntc * 8],
                    ntc * 128, ntc * 128, CH, single_packet=False,
                    queue_num=(gi * 2 + ci) % 2)
            co += ntc * 8

        ind = pool.tile([128, nt, 128], f16, tag=tg + "ind")
        iot_b = T["iot"][:].unsqueeze(1).broadcast_to([128, nt, 128])
        seg_b = T["seg"][:, g["ti"]:g["ti"] + nt].unsqueeze(2)\
            .broadcast_to([128, nt, 128])
        nc.vector.tensor_tensor(ind[:], iot_b, seg_b, A.is_equal)

        # per-edge xr rows via transposed-indicator matmuls (no DMA gather)
        indT = pool.tile([128, nt, 128], f16, tag=tg + "indT")
        xr_sb = pool.tile([128, nt, CV], f16, tag=tg + "xrsb")
        for si, (w, kind, k) in enumerate(g["slots"]):
            psT = xpool.tile([128, 128], f16, tag=tg + "psT",
                             name=f"{tg}psT")
            nc.tensor.transpose(psT[:], ind[:, si, :], T["ident"][:])
            nc.scalar.copy(indT[:, si, :], psT[:])
            xre = xpool.tile([128, CV], f32, tag=tg + "xre",
                             name=f"{tg}xre")
            nc.tensor.matmul(xre[:], indT[:, si, :], xr_acc[:, w, 0:CV],
                             start=True, stop=True, skip_group_check=True)
            nc.scalar.copy(xr_sb[:, si, :], xre[:])

        if MODE == "gather_only":
            return

        xl3 = xlg[:] if layer == 1 else xlg[:, :, 0:64]
        m = spool.tile([128, nt, CV], f16, tag=tg + "m")
        nc.vector.tensor_tensor(m[:], xl3, xr_sb[:], A.add)
        t_ = spool.tile([128, nt, CV], f16, tag=tg + "t")
        nc.vector.scalar_tensor_tensor(t_[:], m[:], SLOPE, m[:],
                                       A.mult, A.max)
        sgn = (T["sg1"] if layer == 1 else T["sg2"])[:].unsqueeze(1)\
            .broadcast_to([128, nt, CV])
        nc.vector.tensor_tensor(m[:], t_[:], sgn, A.mult)
        e = spool.tile([128, nt, NH], f32, tag=tg + "e")
        nc.vector.reduce_sum(
            e[:], m[:].rearrange("p t (h c) -> p t h c", h=NH),
            axis=mybir.AxisListType.X)
        rhs = pool.tile([128, nt, RW], f16, tag=tg + "rhs")
        nc.scalar.activation(rhs[:, :, 0:NH], e[:], AF.Exp)
        wb = rhs[:, :, 0:NH].unsqueeze(3).broadcast_to([128, nt, NH, CPH])
        xl4 = xl3.rearrange("p t (h c) -> p t h c", h=NH)
        nc.vector.tensor_tensor(
            rhs[:, :, NH:RW].rearrange("p t (h c) -> p t h c", h=NH),
            xl4, wb, A.mult)

        pstiles = {}
        last_slot = {}
        for si, (w, kind, k) in enumerate(g["slots"]):
            last_slot[w] = si
        for si, (w, kind, k) in enumerate(g["slots"]):
            st = w not in pstiles
            if st:
                pstiles[w] = ppool.tile([128, RW], f32, tag=tg + "ps",
                                        name=f"{tg}ps{w}")
            nc.tensor.matmul(pstiles[w][:], ind[:, si, :], rhs[:, si, :],
                             start=st, stop=(si == last_slot[w]),
                             skip_group_check=True)

        for w in g["ws"]:
            ps = pstiles[w]
            if layer == 1:
                rec = spool.tile([128, 4], f32, tag=tg + "rec")
                nc.vector.reciprocal(rec[:], ps[:, 0:4])
                u = spool.tile([128, 256], f16, tag=tg + "u")
                nc.vector.tensor_tensor(
                    u[:].rearrange("p (h c) -> p h c", h=4),
                    ps[:, 4:260].rearrange("p (h c) -> p h c", h=4),
                    rec[:].unsqueeze(2).broadcast_to([128, 4, 64]), A.mult)
                nc.vector.tensor_tensor(u[:], u[:], T["rc1"][:], A.mult)
                nc.vector.tensor_tensor(u[:], u[:], T["ba1"][:], A.add)
                lo = spool.tile([128, 256], f16, tag=tg + "lo")
                nc.vector.tensor_scalar_min(lo[:], u[:], 0.0)
                ex = spool.tile([128, 256], f16, tag=tg + "ex")
                nc.scalar.activation(ex[:], lo[:], AF.Exp)
                nc.vector.tensor_scalar_max(u[:], u[:], 0.0)
                hrow = pool.tile([128, 256], f16, tag=tg + "hrow")
                nc.vector.scalar_tensor_tensor(
                    hrow[:], ex[:], -1.0, u[:], A.add, A.add)
                for k in range(2):
                    psH = xpool.tile([128, 128], f16, tag=tg + "psT",
                                     name=f"{tg}psH")
                    nc.tensor.transpose(psH[:], hrow[:, k * 128:(k + 1) * 128],
                                        T["ident"][:])
                    nc.scalar.copy(h[:, k, w * 128:(w + 1) * 128], psH[:])
            else:
                rec = spool.tile([128, 1], f32, tag=tg + "rec2")
                nc.vector.reciprocal(rec[:], ps[:, 0:1])
                u = spool.tile([128, 64], f32, tag=tg + "u2")
                nc.vector.scalar_tensor_tensor(
                    u[:], ps[:, 1:65], rec[:], T["rc2"][:], A.mult, A.mult)
                nc.vector.tensor_tensor(u[:], u[:], T["ba2"][:], A.add)
                nc.sync.dma_start(T["out_d"][w * 128:(w + 1) * 128, :], u[:])


def _host_prep(inputs):
    att1 = np.asarray(inputs["att1"], np.float64)
    att2 = np.asarray(inputs["att2"], np.float64)[0]
    f1 = np.maximum(np.abs(att1.reshape(-1)), 1e-30)
    s1 = np.where(att1.reshape(-1) >= 0, 1.0, -1.0)
    f2 = np.maximum(np.abs(att2), 1e-30)
    s2 = np.where(att2 >= 0, 1.0, -1.0)

    W1 = np.concatenate([np.asarray(inputs["Wl1"], np.float64) * f1,
                         np.asarray(inputs["Wr1"], np.float64) * f1],
                        1)
    b1 = np.concatenate([np.asarray(inputs["bl1"], np.float64) * f1,
                         np.asarray(inputs["br1"], np.float64) * f1])
    W2c = np.concatenate([np.asarray(inputs["Wl2"], np.float64) * f2,
                          np.asarray(inputs["Wr2"], np.float64) * f2],
                         1)                      # [256, 128]
    W2 = np.concatenate([W2c[0:128], W2c[128:256]], 1)  # [128, 256] 2 chunks
    b2 = np.concatenate([np.asarray(inputs["bl2"], np.float64) * f2,
                         np.asarray(inputs["br2"], np.float64) * f2])

    com = dict(
        W1=W1.astype(np.float16),
        b1r=np.tile(b1.reshape(1, 512).astype(np.float16), (128, 1)),
        W2=W2.astype(np.float16),
        b2r=np.tile(b2.reshape(1, 128).astype(np.float16), (128, 1)),
        sg1=np.tile(s1.astype(np.float16), (128, 1)),
        sg2=np.tile(s2.astype(np.float16), (128, 1)),
        rc1=np.tile((1.0 / f1).astype(np.float16), (128, 1)),
        rc2=np.tile((1.0 / f2).astype(np.float32), (128, 1)),
        ba1=np.tile(np.asarray(inputs["bias1"], np.float16), (128, 1)),
        ba2=np.tile(np.asarray(inputs["bias2"], np.float32), (128, 1)),
        iot=np.tile(np.arange(128, dtype=np.int16), (128, 1)),
        ident=np.eye(128, dtype=np.float16),
    )
    x = np.asarray(inputs["x"], np.float32)
    xTs = []
    for c in range(NC):
        xt = np.zeros((128, NPAD), np.float16)
        xt[:, 0:NLOC] = x[c * NLOC:(c + 1) * NLOC].T.astype(np.float16)
        xTs.append(xt)
    return com, xTs


def _get_built(edge_index, rep=1):
    key = (hash(np.asarray(edge_index).tobytes()), rep, MODE)
    if key not in _cache:
        groups, seg_all, idx_all, tot_tiles = _host_metadata(edge_index)
        nc = _build(groups, tot_tiles, rep=rep)
        _cache[key] = (groups, seg_all, idx_all, nc)
    return _cache[key]


def make_maps(inputs, seg_all, idx_all):
    com, xTs = _host_prep(inputs)
    maps = []
    for c in range(NC):
        m = dict(com)
        m["xT"] = xTs[c]
        m["seg"] = seg_all[c]
        m["idxall"] = np.concatenate([a[c] for a in idx_all], axis=1)
        maps.append(m)
    return maps


def kernel(**inputs):
    from concourse.bass_utils import run_bass_kernel_spmd

    groups, seg_all, idx_all, nc = _get_built(inputs["edge_index"])
    maps = make_maps(inputs, seg_all, idx_all)
    res = run_bass_kernel_spmd(nc, maps, list(range(NC)))
    out = np.zeros((N, OUT), np.float32)
    for c in range(NC):
        out[c * NLOC:(c + 1) * NLOC] = res.results[c]["out"][0:NLOC]
    return out



# revision 38
# speedup vs baseline: 172.2424x; 1.0179x over previous
"""GATv2 2-layer GNN on 8 Trainium2 NeuronCores (Bass/Tile).

Sharding: dst-range (6250 dsts/core), edges dst-sorted into 49 windows of
128 dsts. Per-edge endpoint rows are fetched with dma_gather from f16 tables
(512B rows, int16 indices -> tables split at row 25088 into A/B halves).
The |att|-fold plus a +-1 sign mask turns sum_c att_c*lrelu(m_c) into
lrelu + masked reduce. Segment softmax skips max-subtraction (e stays in
[-7, 7] for this model; exp in f16 is safe). Segment sums run on the PE as
0/1-indicator matmuls accumulated into one PSUM tile per window. Layer 2
reuses the same index streams on its own tables. The program is
SPMD-uniform: per-window tile counts are maxed over cores; cores pad with
idx=0 / seg=128 slots which contribute zero.
"""
import sys

sys.path.insert(0, "/opt/trn_rl_repo")

import numpy as np

N = 50000
IN, HID, H, OUT = 128, 64, 4, 64
SLOPE = 0.2
NC = 8
NLOC = N // NC            # 6250
NPAD = 6272               # 49*128
NWIN = NPAD // 128        # 49
NROWS = NPAD * NC         # 50176
SPLIT = NROWS // 2        # 25088
GW = 2                    # windows per gather group
STREAM_MODE = False       # debug: replace gathers with sequential streams
CALL_CAP = 16             # max tiles (x128 idx) per dma_gather call
MODE = "full"             # full | stream | nogather | gather_only | noedge

_cache = {}


def _wrap16(stream):
    n = len(stream)
    a = np.zeros((16, n // 16), np.int16)
    a[np.arange(n) % 16, np.arange(n) // 16] = stream
    return np.tile(a, (8, 1))


def _host_metadata(edge_index):
    src = np.concatenate([np.asarray(edge_index[0], np.int64), np.arange(N)])
    dst = np.concatenate([np.asarray(edge_index[1], np.int64), np.arange(N)])
    srow = (src // NLOC) * NPAD + (src % NLOC)
    dcore = dst // NLOC
    dloc = dst % NLOC

    per_cw = [[None] * NWIN for _ in range(NC)]
    for c in range(NC):
        m = dcore == c
        sr, dl = srow[m], dloc[m]
        o = np.argsort(dl, kind="stable")
        sr, dl = sr[o], dl[o]
        wi = dl // 128
        for w in range(NWIN):
            ws = wi == w
            s_w, d_w = sr[ws], dl[ws] - w * 128
            a = s_w < SPLIT
            per_cw[c][w] = (s_w[a], d_w[a], s_w[~a] - SPLIT, d_w[~a])

    TA = [max((len(per_cw[c][w][0]) + 127) // 128 for c in range(NC))
          for w in range(NWIN)]
    TB = [max((len(per_cw[c][w][2]) + 127) // 128 for c in range(NC))
          for w in range(NWIN)]

    groups = []
    w = 0
    ti = 0
    while w < NWIN:
        ws = list(range(w, min(w + GW, NWIN)))
        na = sum(TA[x] for x in ws)
        nb = sum(TB[x] for x in ws)
        slots = [(x, "A", i) for x in ws for i in range(TA[x])] + \
                [(x, "B", i) for x in ws for i in range(TB[x])]
        t = len(slots)
        calls = []
        for kind, lo, hi in (("A", 0, na), ("B", na, na + nb)):
            s0 = lo
            while s0 < hi:
                n = min(CALL_CAP, hi - s0)
                calls.append((kind, s0, n))
                s0 += n
        groups.append(dict(ws=ws, na=na, nb=nb, slots=slots, calls=calls,
                           cols=sum(n * 8 for _, _, n in calls), ti=ti))
        ti += t
        w += GW
    tot_tiles = ti
    co0 = 0
    for g in groups:
        g["co0"] = co0
        co0 += g["cols"]

    seg_all = np.full((NC, 128, tot_tiles), 128, np.int16)
    idx_all = [np.zeros((NC, 128, g["cols"]), np.int16) for g in groups]
    for gi, g in enumerate(groups):
        nslot = len(g["slots"])
        for c in range(NC):
            slot_idx = np.zeros((nslot, 128), np.int64)
            for si, (w, kind, k) in enumerate(g["slots"]):
                sa, da, sb, db = per_cw[c][w]
                s_, d_ = (sa, da) if kind == "A" else (sb, db)
                iv = s_[k * 128:(k + 1) * 128]
                dv = d_[k * 128:(k + 1) * 128]
                n = len(iv)
                slot_idx[si, :n] = iv
                seg_all[c, :n, g["ti"] + si] = dv
            co = 0
            for kind, s0, ntc in g["calls"]:
                idx_all[gi][c, :, co:co + ntc * 8] = _wrap16(
                    slot_idx[s0:s0 + ntc].reshape(-1))
                co += ntc * 8
    return groups, seg_all, idx_all, tot_tiles


def _build(groups, tot_tiles, rep=1):
    import concourse.bacc as bacc
    import concourse.mybir as mybir
    import concourse.tile as tile
    from concourse import library_config

    f16, f32, i16 = mybir.dt.float16, mybir.dt.float32, mybir.dt.int16
    A = mybir.AluOpType

    nc = bacc.Bacc("TRN2", target_bir_lowering=False, debug=False,
                   num_devices=NC, num_swdge_queues=2)
    totcols = sum(g["cols"] for g in groups)
    d = {}
    for name, shape, dt in [
            ("xT", [128, NPAD], f16), ("W1", [128, 512], f16),
            ("bxr1", [128, 256], f16), ("W2", [128, 256], f16),
            ("bxr2", [128, 64], f16), ("sg1", [128, 256], f16),
            ("sg2", [128, 64], f16), ("rc1", [128, 256], f16),
            ("rc2", [128, 64], f32), ("ba1", [128, 256], f16),
            ("ba2", [128, 64], f32), ("iot", [128, 128], i16),
            ("ident", [128, 128], f16),
            ("idxall", [128, totcols], i16),
            ("seg", [128, tot_tiles], i16)]:
        d[name] = nc.dram_tensor(name, shape, dt, kind="ExternalInput")
    out_d = nc.dram_tensor("out", [NPAD, 64], f32, kind="ExternalOutput")

    with tile.TileContext(nc) as tc:
        with (
            tc.tile_pool(name="const", bufs=1) as cp,
            tc.tile_pool(name="dram", bufs=1, space="DRAM") as dp,
        ):
            nc.gpsimd.load_library(library_config.mlp)
            T = {"out_d": out_d, "dp": dp, "xT_d": d["xT"]}
            for name in d:
                if name == "xT":
                    continue
                tl = cp.tile(list(d[name].shape), d[name].dtype, tag=name)
                nc.sync.dma_start(tl[:], d[name][:, :])
                T[name] = tl

            for r in range(rep):
                _one_pass(nc, tc, mybir, groups, T)
    nc.compile()
    return nc


def _one_pass(nc, tc, mybir, groups, T):
    f16, f32 = mybir.dt.float16, mybir.dt.float32
    A = mybir.AluOpType
    dp = T["dp"]

    xl_loc = dp.tile([NPAD, 256], f16, tag="xl_loc")
    xl_full = dp.tile([NROWS, 256], f16, tag="xl_full",
                      addr_space="Shared")
    l2_loc = dp.tile([NPAD, 128], f16, tag="l2_loc")
    l2_full = dp.tile([NROWS, 128], f16, tag="l2_full",
                      addr_space="Shared")

    with tc.tile_pool(name="acc", bufs=1) as accp:
        # whole-pass SBUF residents: xr/r2 window rows, transposed h, and
        # the gather index streams (shared by both edge layers).
        xracc = accp.tile([128, NWIN, 256], f16)
        r2acc = accp.tile([128, NWIN, 64], f16)
        hTacc = accp.tile([128, 2, NPAD], f16)
        idxs = accp.tile([128, T["idxall"].shape[1]], mybir.dt.int16)
        nc.sync.dma_start(idxs[:], T["idxall"][:])
        T["idxs"] = idxs

        # ---- node phase 1: att-folded xl/xr tables for local nodes ----
        with (
            tc.tile_pool(name="n1", bufs=1) as n1,
            tc.tile_pool(name="n1p", bufs=2, space="PSUM") as n1p,
        ):
            vlacc = n1.tile([128, NWIN, 256], f16)
            xTs = n1.tile([128, NPAD], f16)
            nc.sync.dma_start(xTs[:], T["xT_d"][:, :])
            for t in range(NWIN):
                ps = n1p.tile([128, 512], f32, tag="ps")
                nc.tensor.matmul(ps[:], xTs[:, t * 128:(t + 1) * 128],
                                 T["W1"][:], start=True, stop=True,
                                 skip_group_check=True)
                nc.scalar.copy(vlacc[:, t, :], ps[:, 0:256])
                nc.vector.tensor_tensor(xracc[:, t, :], ps[:, 256:512],
                                        T["bxr1"][:], A.add)
            nc.sync.dma_start(
                xl_loc[:, :].rearrange("(t p) c -> p t c", p=128), vlacc[:])

        nc.gpsimd.collective_compute(
            "AllGather", A.bypass, replica_groups=[list(range(NC))],
            ins=[xl_loc[:].opt()], outs=[xl_full[:].opt()])

        # ---- edge phase 1 -> hT rows in SBUF ----
        if MODE != "noedge":
            with (
                tc.tile_pool(name="e1", bufs=2) as e1,
                tc.tile_pool(name="e1s", bufs=1) as e1s,
                tc.tile_pool(name="e1p", bufs=2, space="PSUM") as e1p,
                tc.tile_pool(name="e1px", bufs=2, space="PSUM") as e1px,
            ):
                _edge_layer(nc, tc, mybir, groups, T, e1, e1s, e1p, e1px,
                            layer=1, src_tab=xl_full, xr_acc=xracc,
                            h=hTacc)

            # ---- node phase 2: hl2/hr2 tables ----
            if MODE == "gather_only":
                return
            with (
                tc.tile_pool(name="n2", bufs=1) as n2,
                tc.tile_pool(name="n2p", bufs=2, space="PSUM") as n2p,
            ):
                v2acc = n2.tile([128, NWIN, 128], f16)
                for t in range(NWIN):
                    ps = n2p.tile([128, 128], f32, tag="ps2")
                    for k in range(2):
                        nc.tensor.matmul(
                            ps[:], hTacc[:, k, t * 128:(t + 1) * 128],
                            T["W2"][:, k * 128:(k + 1) * 128],
                            start=(k == 0), stop=(k == 1),
                            skip_group_check=True)
                    nc.vector.memset(v2acc[:, t, 64:128], 0.0)
                    nc.scalar.copy(v2acc[:, t, 0:64], ps[:, 0:64])
                    nc.vector.tensor_tensor(r2acc[:, t, :], ps[:, 64:128],
                                            T["bxr2"][:], A.add)
                nc.sync.dma_start(
                    l2_loc[:, :].rearrange("(t p) c -> p t c", p=128),
                    v2acc[:])

        nc.gpsimd.collective_compute(
            "AllGather", A.bypass, replica_groups=[list(range(NC))],
            ins=[l2_loc[:].opt()], outs=[l2_full[:].opt()])

        # ---- edge phase 2 -> output rows ----
        if MODE not in ("noedge", "gather_only"):
            with (
                tc.tile_pool(name="e2", bufs=2) as e2,
                tc.tile_pool(name="e2s", bufs=1) as e2s,
                tc.tile_pool(name="e2p", bufs=2, space="PSUM") as e2p,
                tc.tile_pool(name="e2px", bufs=2, space="PSUM") as e2px,
            ):
                _edge_layer(nc, tc, mybir, groups, T, e2, e2s, e2p, e2px,
                            layer=2, src_tab=l2_full, xr_acc=r2acc, h=None)


def _edge_layer(nc, tc, mybir, groups, T, pool, spool, ppool, xpool, layer,
                src_tab, xr_acc, h):
    f16, f32 = mybir.dt.float16, mybir.dt.float32
    A = mybir.AluOpType
    AF = mybir.ActivationFunctionType
    CH = 256 if layer == 1 else 128   # gathered row width (f16 elems)
    CV = 256 if layer == 1 else 64    # valid channels
    NH = 4 if layer == 1 else 1       # heads
    CPH = CV // NH                    # channels per head
    RW = NH + CV                      # [w | wfeat]
    tg = f"l{layer}"

    for gi, g in enumerate(groups):
        nt = len(g["slots"])
        idxs = T["idxs"]
        xlg = pool.tile([128, nt, CH], f16, tag=tg + "xl")
        co = g["co0"]
        for ci, (kind, s0, ntc) in enumerate(g["calls"]):
            src = src_tab[0:SPLIT, :] if kind == "A" else src_tab[SPLIT:NROWS, :]
            if MODE == "nogather":
                nc.vector.memset(xlg[:, s0:s0 + ntc, 0:8], 0.0)
            elif STREAM_MODE or MODE == "stream":
                r0 = (g["ti"] * 64) % 4096
                nc.sync.dma_start(
                    xlg[:, s0:s0 + ntc, :],
                    src[r0:r0 + ntc * 128, :].rearrange(
                        "(t p) c -> p t c", p=128))
            else:
                nc.gpsimd.dma_gather(
                    xlg[:, s0:s0 + ntc, :], src, idxs[:, co:co + ntc * 8],
                    ntc * 128, ntc * 128, CH, single_packet=False,
                    queue_num=(gi * 2 + ci) % 2)
            co += ntc * 8

        ind = pool.tile([128, nt, 128], f16, tag=tg + "ind")
        iot_b = T["iot"][:].unsqueeze(1).broadcast_to([128, nt, 128])
        seg_b = T["seg"][:, g["ti"]:g["ti"] + nt].unsqueeze(2)\
            .broadcast_to([128, nt, 128])
        nc.vector.tensor_tensor(ind[:], iot_b, seg_b, A.is_equal)

        # per-edge xr rows via transposed-indicator matmuls (no DMA gather);
        # transposes/matmuls land in quad-batched PSUM tiles so one scalar
        # copy moves four tiles' worth.
        indT = pool.tile([128, nt, 128], f16, tag=tg + "indT")
        xr_sb = pool.tile([128, nt, CV], f16, tag=tg + "xrsb")
        for si, (w, kind, k) in enumerate(g["slots"]):
            psT = xpool.tile([128, 128], f16, tag=tg + "psT",
                             name=f"{tg}psT")
            nc.tensor.transpose(psT[:], ind[:, si, :], T["ident"][:])
            nc.scalar.copy(indT[:, si, :], psT[:])
            xre = xpool.tile([128, CV], f32, tag=tg + "xre",
                             name=f"{tg}xre")
            nc.tensor.matmul(xre[:], indT[:, si, :], xr_acc[:, w, 0:CV],
                             start=True, stop=True, skip_group_check=True)
            nc.scalar.copy(xr_sb[:, si, :], xre[:])

        if MODE == "gather_only":
            return

        xl3 = xlg[:] if layer == 1 else xlg[:, :, 0:64]
        m = spool.tile([128, nt, CV], f16, tag=tg + "m")
        nc.vector.tensor_tensor(m[:], xl3, xr_sb[:], A.add)
        t_ = spool.tile([128, nt, CV], f16, tag=tg + "t")
        nc.vector.scalar_tensor_tensor(t_[:], m[:], SLOPE, m[:],
                                       A.mult, A.max)
        sgn = (T["sg1"] if layer == 1 else T["sg2"])[:].unsqueeze(1)\
            .broadcast_to([128, nt, CV])
        nc.vector.tensor_tensor(m[:], t_[:], sgn, A.mult)
        e = spool.tile([128, nt, NH], f32, tag=tg + "e")
        nc.vector.reduce_sum(
            e[:], m[:].rearrange("p t (h c) -> p t h c", h=NH),
            axis=mybir.AxisListType.X)
        rhs = pool.tile([128, nt, RW], f16, tag=tg + "rhs")
        nc.scalar.activation(rhs[:, :, 0:NH], e[:], AF.Exp)
        wb = rhs[:, :, 0:NH].unsqueeze(3).broadcast_to([128, nt, NH, CPH])
        xl4 = xl3.rearrange("p t (h c) -> p t h c", h=NH)
        nc.vector.tensor_tensor(
            rhs[:, :, NH:RW].rearrange("p t (h c) -> p t h c", h=NH),
            xl4, wb, A.mult)

        pstiles = {}
        last_slot = {}
        for si, (w, kind, k) in enumerate(g["slots"]):
            last_slot[w] = si
        for si, (w, kind, k) in enumerate(g["slots"]):
            st = w not in pstiles
            if st:
                pstiles[w] = ppool.tile([128, RW], f32, tag=tg + "ps",
                                        name=f"{tg}ps{w}")
            nc.tensor.matmul(pstiles[w][:], ind[:, si, :], rhs[:, si, :],
                             start=st, stop=(si == last_slot[w]),
                             skip_group_check=True)

        for w in g["ws"]:
            ps = pstiles[w]
            if layer == 1:
                rec = spool.tile([128, 4], f32, tag=tg + "rec")
                nc.vector.reciprocal(rec[:], ps[:, 0:4])
                u = spool.tile([128, 256], f16, tag=tg + "u")
                nc.vector.tensor_tensor(
                    u[:].rearrange("p (h c) -> p h c", h=4),
                    ps[:, 4:260].rearrange("p (h c) -> p h c", h=4),
                    rec[:].unsqueeze(2).broadcast_to([128, 4, 64]), A.mult)
                nc.vector.tensor_tensor(u[:], u[:], T["rc1"][:], A.mult)
                nc.vector.tensor_tensor(u[:], u[:], T["ba1"][:], A.add)
                lo = spool.tile([128, 256], f16, tag=tg + "lo")
                nc.vector.tensor_scalar_min(lo[:], u[:], 0.0)
                ex = spool.tile([128, 256], f16, tag=tg + "ex")
                nc.scalar.activation(ex[:], lo[:], AF.Exp)
                nc.vector.tensor_scalar_max(u[:], u[:], 0.0)
                hrow = pool.tile([128, 256], f16, tag=tg + "hrow")
                nc.vector.scalar_tensor_tensor(
                    hrow[:], ex[:], -1.0, u[:], A.add, A.add)
                for k in range(2):
                    psH = xpool.tile([128, 128], f16, tag=tg + "psT",
                                     name=f"{tg}psH")
                    nc.tensor.transpose(psH[:],
                                        hrow[:, k * 128:(k + 1) * 128],
                                        T["ident"][:])
                    nc.scalar.copy(h[:, k, w * 128:(w + 1) * 128], psH[:])
            else:
                rec = spool.tile([128, 1], f32, tag=tg + "rec2")
                nc.vector.reciprocal(rec[:], ps[:, 0:1])
                u = spool.tile([128, 64], f32, tag=tg + "u2")
                nc.vector.scalar_tensor_tensor(
                    u[:], ps[:, 1:65], rec[:], T["rc2"][:], A.mult, A.mult)
                nc.vector.tensor_tensor(u[:], u[:], T["ba2"][:], A.add)
                nc.sync.dma_start(T["out_d"][w * 128:(w + 1) * 128, :], u[:])


def _host_prep(inputs):
    att1 = np.asarray(inputs["att1"], np.float64)
    att2 = np.asarray(inputs["att2"], np.float64)[0]
    f1 = np.maximum(np.abs(att1.reshape(-1)), 1e-30)
    s1 = np.where(att1.reshape(-1) >= 0, 1.0, -1.0)
    f2 = np.maximum(np.abs(att2), 1e-30)
    s2 = np.where(att2 >= 0, 1.0, -1.0)

    W1 = np.concatenate([np.asarray(inputs["Wl1"], np.float64) * f1,
                         np.asarray(inputs["Wr1"], np.float64) * f1],
                        1)
    b1 = np.concatenate([np.asarray(inputs["bl1"], np.float64) * f1,
                         np.asarray(inputs["br1"], np.float64) * f1])
    W2c = np.concatenate([np.asarray(inputs["Wl2"], np.float64) * f2,
                          np.asarray(inputs["Wr2"], np.float64) * f2],
                         1)                      # [256, 128]
    W2 = np.concatenate([W2c[0:128], W2c[128:256]], 1)  # [128, 256] 2 chunks
    b2 = np.concatenate([np.asarray(inputs["bl2"], np.float64) * f2,
                         np.asarray(inputs["br2"], np.float64) * f2])

    bxr1 = (b1[0:256] + b1[256:512]).astype(np.float16)
    bxr2 = (b2[0:64] + b2[64:128]).astype(np.float16)
    bl1 = np.asarray(inputs["bl1"], np.float64)
    bl2 = np.asarray(inputs["bl2"], np.float64)
    com = dict(
        W1=W1.astype(np.float16),
        bxr1=np.tile(bxr1, (128, 1)),
        W2=W2.astype(np.float16),
        bxr2=np.tile(bxr2, (128, 1)),
        sg1=np.tile(s1.astype(np.float16), (128, 1)),
        sg2=np.tile(s2.astype(np.float16), (128, 1)),
        rc1=np.tile((1.0 / f1).astype(np.float16), (128, 1)),
        rc2=np.tile((1.0 / f2).astype(np.float32), (128, 1)),
        ba1=np.tile((np.asarray(inputs["bias1"], np.float64) + bl1)
                    .astype(np.float16), (128, 1)),
        ba2=np.tile((np.asarray(inputs["bias2"], np.float64) + bl2)
                    .astype(np.float32), (128, 1)),
        iot=np.tile(np.arange(128, dtype=np.int16), (128, 1)),
        ident=np.eye(128, dtype=np.float16),
    )
    x = np.asarray(inputs["x"], np.float32)
    xTs = []
    for c in range(NC):
        xt = np.zeros((128, NPAD), np.float16)
        xt[:, 0:NLOC] = x[c * NLOC:(c + 1) * NLOC].T.astype(np.float16)
        xTs.append(xt)
    return com, xTs


def _get_built(edge_index, rep=1):
    key = (hash(np.asarray(edge_index).tobytes()), rep, MODE)
    if key not in _cache:
        groups, seg_all, idx_all, tot_tiles = _host_metadata(edge_index)
        nc = _build(groups, tot_tiles, rep=rep)
        _cache[key] = (groups, seg_all, idx_all, nc)
    return _cache[key]


def make_maps(inputs, seg_all, idx_all):
    com, xTs = _host_prep(inputs)
    maps = []
    for c in range(NC):
        m = dict(com)
        m["xT"] = xTs[c]
        m["seg"] = seg_all[c]
        m["idxall"] = np.concatenate([a[c] for a in idx_all], axis=1)
        maps.append(m)
    return maps


def kernel(**inputs):
    from concourse.bass_utils import run_bass_kernel_spmd

    groups, seg_all, idx_all, nc = _get_built(inputs["edge_index"])
    maps = make_maps(inputs, seg_all, idx_all)
    res = run_bass_kernel_spmd(nc, maps, list(range(NC)))
    out = np.zeros((N, OUT), np.float32)
    for c in range(NC):
        out[c * NLOC:(c + 1) * NLOC] = res.results[c]["out"][0:NLOC]
    return out



# revision 40
# speedup vs baseline: 174.4389x; 1.0128x over previous
"""GATv2 2-layer GNN on 8 Trainium2 NeuronCores (Bass/Tile).

Sharding: dst-range (6250 dsts/core), edges dst-sorted into 49 windows of
128 dsts. Per-edge endpoint rows are fetched with dma_gather from f16 tables
(512B rows, int16 indices -> tables split at row 25088 into A/B halves).
The |att|-fold plus a +-1 sign mask turns sum_c att_c*lrelu(m_c) into
lrelu + masked reduce. Segment softmax skips max-subtraction (e stays in
[-7, 7] for this model; exp in f16 is safe). Segment sums run on the PE as
0/1-indicator matmuls accumulated into one PSUM tile per window. Layer 2
reuses the same index streams on its own tables. The program is
SPMD-uniform: per-window tile counts are maxed over cores; cores pad with
idx=0 / seg=128 slots which contribute zero.
"""
import sys

sys.path.insert(0, "/opt/trn_rl_repo")

import numpy as np

N = 50000
IN, HID, H, OUT = 128, 64, 4, 64
SLOPE = 0.2
NC = 8
NLOC = N // NC            # 6250
NPAD = 6272               # 49*128
NWIN = NPAD // 128        # 49
NROWS = NPAD * NC         # 50176
SPLIT = NROWS // 2        # 25088
GW = 2                    # windows per gather group
STREAM_MODE = False       # debug: replace gathers with sequential streams
CALL_CAP = 16             # max tiles (x128 idx) per dma_gather call
MODE = "full"             # full | stream | nogather | gather_only | noedge

_cache = {}


def _wrap16(stream):
    n = len(stream)
    a = np.zeros((16, n // 16), np.int16)
    a[np.arange(n) % 16, np.arange(n) // 16] = stream
    return np.tile(a, (8, 1))


def _host_metadata(edge_index):
    src = np.concatenate([np.asarray(edge_index[0], np.int64), np.arange(N)])
    dst = np.concatenate([np.asarray(edge_index[1], np.int64), np.arange(N)])
    srow = (src // NLOC) * NPAD + (src % NLOC)
    dcore = dst // NLOC
    dloc = dst % NLOC

    per_cw = [[None] * NWIN for _ in range(NC)]
    for c in range(NC):
        m = dcore == c
        sr, dl = srow[m], dloc[m]
        o = np.argsort(dl, kind="stable")
        sr, dl = sr[o], dl[o]
        wi = dl // 128
        for w in range(NWIN):
            ws = wi == w
            s_w, d_w = sr[ws], dl[ws] - w * 128
            a = s_w < SPLIT
            per_cw[c][w] = (s_w[a], d_w[a], s_w[~a] - SPLIT, d_w[~a])

    TA = [max((len(per_cw[c][w][0]) + 127) // 128 for c in range(NC))
          for w in range(NWIN)]
    TB = [max((len(per_cw[c][w][2]) + 127) // 128 for c in range(NC))
          for w in range(NWIN)]

    groups = []
    w = 0
    ti = 0
    while w < NWIN:
        ws = list(range(w, min(w + GW, NWIN)))
        na = sum(TA[x] for x in ws)
        nb = sum(TB[x] for x in ws)
        slots = [(x, "A", i) for x in ws for i in range(TA[x])] + \
                [(x, "B", i) for x in ws for i in range(TB[x])]
        t = len(slots)
        calls = []
        for kind, lo, hi in (("A", 0, na), ("B", na, na + nb)):
            s0 = lo
            while s0 < hi:
                n = min(CALL_CAP, hi - s0)
                calls.append((kind, s0, n))
                s0 += n
        groups.append(dict(ws=ws, na=na, nb=nb, slots=slots, calls=calls,
                           cols=sum(n * 8 for _, _, n in calls), ti=ti))
        ti += t
        w += GW
    tot_tiles = ti
    co0 = 0
    for g in groups:
        g["co0"] = co0
        co0 += g["cols"]

    seg_all = np.full((NC, 128, tot_tiles), 128, np.int16)
    idx_all = [np.zeros((NC, 128, g["cols"]), np.int16) for g in groups]
    for gi, g in enumerate(groups):
        nslot = len(g["slots"])
        for c in range(NC):
            slot_idx = np.zeros((nslot, 128), np.int64)
            for si, (w, kind, k) in enumerate(g["slots"]):
                sa, da, sb, db = per_cw[c][w]
                s_, d_ = (sa, da) if kind == "A" else (sb, db)
                iv = s_[k * 128:(k + 1) * 128]
                dv = d_[k * 128:(k + 1) * 128]
                n = len(iv)
                slot_idx[si, :n] = iv
                seg_all[c, :n, g["ti"] + si] = dv
            co = 0
            for kind, s0, ntc in g["calls"]:
                idx_all[gi][c, :, co:co + ntc * 8] = _wrap16(
                    slot_idx[s0:s0 + ntc].reshape(-1))
                co += ntc * 8
    return groups, seg_all, idx_all, tot_tiles


def _build(groups, tot_tiles, rep=1):
    import concourse.bacc as bacc
    import concourse.mybir as mybir
    import concourse.tile as tile
    from concourse import library_config

    f16, f32, i16 = mybir.dt.float16, mybir.dt.float32, mybir.dt.int16
    A = mybir.AluOpType

    nc = bacc.Bacc("TRN2", target_bir_lowering=False, debug=False,
                   num_devices=NC, num_swdge_queues=2)
    totcols = sum(g["cols"] for g in groups)
    d = {}
    for name, shape, dt in [
            ("xT", [128, NPAD], f16), ("W1", [128, 512], f16),
            ("bxr1", [128, 256], f16), ("W2", [128, 256], f16),
            ("bxr2", [128, 64], f16), ("sg1", [128, 256], f16),
            ("sg2", [128, 64], f16), ("rc1", [128, 256], f16),
            ("rc2", [128, 64], f32), ("ba1", [128, 256], f16),
            ("ba2", [128, 64], f32), ("iot", [128, 128], i16),
            ("ident", [128, 128], f16),
            ("idxall", [128, totcols], i16),
            ("seg", [128, tot_tiles], i16)]:
        d[name] = nc.dram_tensor(name, shape, dt, kind="ExternalInput")
    out_d = nc.dram_tensor("out", [NPAD, 64], f32, kind="ExternalOutput")

    with tile.TileContext(nc) as tc:
        with (
            tc.tile_pool(name="const", bufs=1) as cp,
            tc.tile_pool(name="dram", bufs=1, space="DRAM") as dp,
        ):
            nc.gpsimd.load_library(library_config.mlp)
            T = {"out_d": out_d, "dp": dp, "xT_d": d["xT"]}
            for name in d:
                if name == "xT":
                    continue
                tl = cp.tile(list(d[name].shape), d[name].dtype, tag=name)
                nc.sync.dma_start(tl[:], d[name][:, :])
                T[name] = tl

            for r in range(rep):
                _one_pass(nc, tc, mybir, groups, T)
    nc.compile()
    return nc


def _one_pass(nc, tc, mybir, groups, T):
    f16, f32 = mybir.dt.float16, mybir.dt.float32
    A = mybir.AluOpType
    dp = T["dp"]

    xl_loc = dp.tile([NPAD, 256], f16, tag="xl_loc")
    xl_full = dp.tile([NROWS, 256], f16, tag="xl_full",
                      addr_space="Shared")
    l2_loc = dp.tile([NPAD, 128], f16, tag="l2_loc")
    l2_full = dp.tile([NROWS, 128], f16, tag="l2_full",
                      addr_space="Shared")

    with tc.tile_pool(name="acc", bufs=1) as accp:
        # whole-pass SBUF residents: xr/r2 window rows, transposed h, and
        # the gather index streams (shared by both edge layers).
        xracc = accp.tile([128, NWIN, 256], f16)
        r2acc = accp.tile([128, NWIN, 64], f16)
        hTacc = accp.tile([128, 2, NPAD], f16)
        idxs = accp.tile([128, T["idxall"].shape[1]], mybir.dt.int16)
        nc.sync.dma_start(idxs[:], T["idxall"][:])
        T["idxs"] = idxs

        # ---- node phase 1: att-folded xl/xr tables for local nodes ----
        with (
            tc.tile_pool(name="n1", bufs=1) as n1,
            tc.tile_pool(name="n1p", bufs=2, space="PSUM") as n1p,
        ):
            vlacc = n1.tile([128, NWIN, 256], f16)
            xTs = n1.tile([128, NPAD], f16)
            nc.sync.dma_start(xTs[:], T["xT_d"][:, :])
            for t in range(NWIN):
                ps = n1p.tile([128, 512], f32, tag="ps")
                nc.tensor.matmul(ps[:], xTs[:, t * 128:(t + 1) * 128],
                                 T["W1"][:], start=True, stop=True,
                                 skip_group_check=True)
                nc.scalar.copy(vlacc[:, t, :], ps[:, 0:256])
                nc.vector.tensor_tensor(xracc[:, t, :], ps[:, 256:512],
                                        T["bxr1"][:], A.add)
            nc.sync.dma_start(
                xl_loc[:, :].rearrange("(t p) c -> p t c", p=128), vlacc[:])

        nc.gpsimd.collective_compute(
            "AllGather", A.bypass, replica_groups=[list(range(NC))],
            ins=[xl_loc[:].opt()], outs=[xl_full[:].opt()])

        # ---- edge phase 1 -> hT rows in SBUF ----
        if MODE != "noedge":
            with (
                tc.tile_pool(name="e1", bufs=2) as e1,
                tc.tile_pool(name="e1s", bufs=1) as e1s,
                tc.tile_pool(name="e1p", bufs=2, space="PSUM") as e1p,
                tc.tile_pool(name="e1px", bufs=2, space="PSUM") as e1px,
            ):
                _edge_layer(nc, tc, mybir, groups, T, e1, e1s, e1p, e1px,
                            layer=1, src_tab=xl_full, xr_acc=xracc,
                            h=hTacc)

            # ---- node phase 2: hl2/hr2 tables ----
            if MODE == "gather_only":
                return
            with (
                tc.tile_pool(name="n2", bufs=1) as n2,
                tc.tile_pool(name="n2p", bufs=2, space="PSUM") as n2p,
            ):
                v2acc = n2.tile([128, NWIN, 128], f16)
                for t in range(NWIN):
                    ps = n2p.tile([128, 128], f32, tag="ps2")
                    for k in range(2):
                        nc.tensor.matmul(
                            ps[:], hTacc[:, k, t * 128:(t + 1) * 128],
                            T["W2"][:, k * 128:(k + 1) * 128],
                            start=(k == 0), stop=(k == 1),
                            skip_group_check=True)
                    nc.vector.memset(v2acc[:, t, 64:128], 0.0)
                    nc.scalar.copy(v2acc[:, t, 0:64], ps[:, 0:64])
                    nc.vector.tensor_tensor(r2acc[:, t, :], ps[:, 64:128],
                                            T["bxr2"][:], A.add)
                nc.sync.dma_start(
                    l2_loc[:, :].rearrange("(t p) c -> p t c", p=128),
                    v2acc[:])

        nc.gpsimd.collective_compute(
            "AllGather", A.bypass, replica_groups=[list(range(NC))],
            ins=[l2_loc[:].opt()], outs=[l2_full[:].opt()])

        # ---- edge phase 2 -> output rows ----
        if MODE not in ("noedge", "gather_only"):
            with (
                tc.tile_pool(name="e2", bufs=2) as e2,
                tc.tile_pool(name="e2s", bufs=1) as e2s,
                tc.tile_pool(name="e2p", bufs=2, space="PSUM") as e2p,
                tc.tile_pool(name="e2px", bufs=2, space="PSUM") as e2px,
            ):
                _edge_layer(nc, tc, mybir, groups, T, e2, e2s, e2p, e2px,
                            layer=2, src_tab=l2_full, xr_acc=r2acc, h=None)


def _edge_layer(nc, tc, mybir, groups, T, pool, spool, ppool, xpool, layer,
                src_tab, xr_acc, h):
    f16, f32 = mybir.dt.float16, mybir.dt.float32
    A = mybir.AluOpType
    AF = mybir.ActivationFunctionType
    CH = 256 if layer == 1 else 128   # gathered row width (f16 elems)
    CV = 256 if layer == 1 else 64    # valid channels
    NH = 4 if layer == 1 else 1       # heads
    CPH = CV // NH                    # channels per head
    RW = NH + CV                      # [w | wfeat]
    tg = f"l{layer}"

    for gi, g in enumerate(groups):
        nt = len(g["slots"])
        idxs = T["idxs"]
        xlg = pool.tile([128, nt, CH], f16, tag=tg + "xl")
        co = g["co0"]
        for ci, (kind, s0, ntc) in enumerate(g["calls"]):
            src = src_tab[0:SPLIT, :] if kind == "A" else src_tab[SPLIT:NROWS, :]
            if MODE == "nogather":
                nc.vector.memset(xlg[:, s0:s0 + ntc, 0:8], 0.0)
            elif STREAM_MODE or MODE == "stream":
                r0 = (g["ti"] * 64) % 4096
                nc.sync.dma_start(
                    xlg[:, s0:s0 + ntc, :],
                    src[r0:r0 + ntc * 128, :].rearrange(
                        "(t p) c -> p t c", p=128))
            else:
                nc.gpsimd.dma_gather(
                    xlg[:, s0:s0 + ntc, :], src, idxs[:, co:co + ntc * 8],
                    ntc * 128, ntc * 128, CH, single_packet=False,
                    queue_num=(gi * 2 + ci) % 2)
            co += ntc * 8

        ind = pool.tile([128, nt, 128], f16, tag=tg + "ind")
        iot_b = T["iot"][:].unsqueeze(1).broadcast_to([128, nt, 128])
        seg_b = T["seg"][:, g["ti"]:g["ti"] + nt].unsqueeze(2)\
            .broadcast_to([128, nt, 128])
        nc.vector.tensor_tensor(ind[:], iot_b, seg_b, A.is_equal)

        # per-edge xr rows via transposed-indicator matmuls (no DMA gather);
        # transposes/matmuls land in quad-batched PSUM tiles so one scalar
        # copy moves four tiles' worth.
        indT = pool.tile([128, nt, 128], f16, tag=tg + "indT")
        xr_sb = pool.tile([128, nt, CV], f16, tag=tg + "xrsb")
        for si, (w, kind, k) in enumerate(g["slots"]):
            psT = xpool.tile([128, 128], f16, tag=tg + "psT",
                             name=f"{tg}psT")
            nc.tensor.transpose(psT[:], ind[:, si, :], T["ident"][:])
            nc.scalar.copy(indT[:, si, :], psT[:])
            xre = xpool.tile([128, CV], f32, tag=tg + "xre",
                             name=f"{tg}xre")
            nc.tensor.matmul(xre[:], indT[:, si, :], xr_acc[:, w, 0:CV],
                             start=True, stop=True, skip_group_check=True)
            nc.scalar.copy(xr_sb[:, si, :], xre[:])

        if MODE == "gather_only":
            return

        xl3 = xlg[:] if layer == 1 else xlg[:, :, 0:64]
        m = spool.tile([128, nt, CV], f16, tag=tg + "m")
        nc.vector.tensor_tensor(m[:], xl3, xr_sb[:], A.add)
        t_ = spool.tile([128, nt, CV], f16, tag=tg + "t")
        nc.vector.scalar_tensor_tensor(t_[:], m[:], SLOPE, m[:],
                                       A.mult, A.max)
        sgn = (T["sg1"] if layer == 1 else T["sg2"])[:].unsqueeze(1)\
            .broadcast_to([128, nt, CV])
        nc.vector.tensor_tensor(m[:], t_[:], sgn, A.mult)
        e = spool.tile([128, nt, NH], f32, tag=tg + "e")
        nc.vector.reduce_sum(
            e[:], m[:].rearrange("p t (h c) -> p t h c", h=NH),
            axis=mybir.AxisListType.X)
        rhs = pool.tile([128, nt, RW], f16, tag=tg + "rhs")
        nc.scalar.activation(rhs[:, :, 0:NH], e[:], AF.Exp)
        wb = rhs[:, :, 0:NH].unsqueeze(3).broadcast_to([128, nt, NH, CPH])
        xl4 = xl3.rearrange("p t (h c) -> p t h c", h=NH)
        nc.vector.tensor_tensor(
            rhs[:, :, NH:RW].rearrange("p t (h c) -> p t h c", h=NH),
            xl4, wb, A.mult)

        pstiles = {}
        last_slot = {}
        for si, (w, kind, k) in enumerate(g["slots"]):
            last_slot[w] = si
        for si, (w, kind, k) in enumerate(g["slots"]):
            st = w not in pstiles
            if st:
                pstiles[w] = ppool.tile([128, RW], f32, tag=tg + "ps",
                                        name=f"{tg}ps{w}")
            nc.tensor.matmul(pstiles[w][:], ind[:, si, :], rhs[:, si, :],
                             start=st, stop=(si == last_slot[w]),
                             skip_group_check=True)

        for w in g["ws"]:
            ps = pstiles[w]
            if layer == 1:
                rec = spool.tile([128, 4], f32, tag=tg + "rec")
                nc.vector.reciprocal(rec[:], ps[:, 0:4])
                u = spool.tile([128, 256], f16, tag=tg + "u")
                nc.vector.tensor_tensor(
                    u[:].rearrange("p (h c) -> p h c", h=4),
                    ps[:, 4:260].rearrange("p (h c) -> p h c", h=4),
                    rec[:].unsqueeze(2).broadcast_to([128, 4, 64]), A.mult)
                nc.vector.tensor_tensor(u[:], u[:], T["rc1"][:], A.mult)
                nc.vector.tensor_tensor(u[:], u[:], T["ba1"][:], A.add)
                lo = spool.tile([128, 256], f16, tag=tg + "lo")
                nc.vector.tensor_scalar_min(lo[:], u[:], 0.0)
                ex = spool.tile([128, 256], f16, tag=tg + "ex")
                nc.scalar.activation(ex[:], lo[:], AF.Exp)
                nc.vector.tensor_scalar_max(u[:], u[:], 0.0)
                hrow = pool.tile([128, 256], f16, tag=tg + "hrow")
                nc.vector.scalar_tensor_tensor(
                    hrow[:], ex[:], -1.0, u[:], A.add, A.add)
                for k in range(2):
                    psH = xpool.tile([128, 128], f16, tag=tg + "psT",
                                     name=f"{tg}psH")
                    nc.tensor.transpose(psH[:],
                                        hrow[:, k * 128:(k + 1) * 128],
                                        T["ident"][:])
                    nc.scalar.copy(h[:, k, w * 128:(w + 1) * 128], psH[:])
            else:
                rec = spool.tile([128, 1], f32, tag=tg + "rec2")
                nc.vector.reciprocal(rec[:], ps[:, 0:1])
                u = spool.tile([128, 64], f32, tag=tg + "u2")
                nc.vector.scalar_tensor_tensor(
                    u[:], ps[:, 1:65], rec[:], T["rc2"][:], A.mult, A.mult)
                nc.vector.tensor_tensor(u[:], u[:], T["ba2"][:], A.add)
                nc.sync.dma_start(T["out_d"][w * 128:(w + 1) * 128, :], u[:])


def _host_prep(inputs):
    att1 = np.asarray(inputs["att1"], np.float64)
    att2 = np.asarray(inputs["att2"], np.float64)[0]
    f1 = np.maximum(np.abs(att1.reshape(-1)), 1e-30)
    s1 = np.where(att1.reshape(-1) >= 0, 1.0, -1.0)
    f2 = np.maximum(np.abs(att2), 1e-30)
    s2 = np.where(att2 >= 0, 1.0, -1.0)

    W1 = np.concatenate([np.asarray(inputs["Wl1"], np.float64) * f1,
                         np.asarray(inputs["Wr1"], np.float64) * f1],
                        1)
    b1 = np.concatenate([np.asarray(inputs["bl1"], np.float64) * f1,
                         np.asarray(inputs["br1"], np.float64) * f1])
    W2c = np.concatenate([np.asarray(inputs["Wl2"], np.float64) * f2,
                          np.asarray(inputs["Wr2"], np.float64) * f2],
                         1)                      # [256, 128]
    W2 = np.concatenate([W2c[0:128], W2c[128:256]], 1)  # [128, 256] 2 chunks
    b2 = np.concatenate([np.asarray(inputs["bl2"], np.float64) * f2,
                         np.asarray(inputs["br2"], np.float64) * f2])

    bxr1 = (b1[0:256] + b1[256:512]).astype(np.float16)
    bxr2 = (b2[0:64] + b2[64:128]).astype(np.float16)
    bl1 = np.asarray(inputs["bl1"], np.float64)
    bl2 = np.asarray(inputs["bl2"], np.float64)
    com = dict(
        W1=W1.astype(np.float16),
        bxr1=np.tile(bxr1, (128, 1)),
        W2=W2.astype(np.float16),
        bxr2=np.tile(bxr2, (128, 1)),
        sg1=np.tile(s1.astype(np.float16), (128, 1)),
        sg2=np.tile(s2.astype(np.float16), (128, 1)),
        rc1=np.tile((1.0 / f1).astype(np.float16), (128, 1)),
        rc2=np.tile((1.0 / f2).astype(np.float32), (128, 1)),
        ba1=np.tile((np.asarray(inputs["bias1"], np.float64) + bl1)
                    .astype(np.float16), (128, 1)),
        ba2=np.tile((np.asarray(inputs["bias2"], np.float64) + bl2)
                    .astype(np.float32), (128, 1)),
        iot=np.tile(np.arange(128, dtype=np.int16), (128, 1)),
        ident=np.eye(128, dtype=np.float16),
    )
    x = np.asarray(inputs["x"], np.float32)
    xTs = []
    for c in range(NC):
        xt = np.zeros((128, NPAD), np.float16)
        xt[:, 0:NLOC] = x[c * NLOC:(c + 1) * NLOC].T.astype(np.float16)
        xTs.append(xt)
    return com, xTs


def _get_built(edge_index, rep=1):
    key = (hash(np.asarray(edge_index).tobytes()), rep, MODE)
    if key not in _cache:
        groups, seg_all, idx_all, tot_tiles = _host_metadata(edge_index)
        nc = _build(groups, tot_tiles, rep=rep)
        _cache[key] = (groups, seg_all, idx_all, nc)
    return _cache[key]


def make_maps(inputs, seg_all, idx_all):
    com, xTs = _host_prep(inputs)
    maps = []
    for c in range(NC):
        m = dict(com)
        m["xT"] = xTs[c]
        m["seg"] = seg_all[c]
        m["idxall"] = np.concatenate([a[c] for a in idx_all], axis=1)
        maps.append(m)
    return maps


def kernel(**inputs):
    from concourse.bass_utils import run_bass_kernel_spmd

    groups, seg_all, idx_all, nc = _get_built(inputs["edge_index"])
    maps = make_maps(inputs, seg_all, idx_all)
    res = run_bass_kernel_spmd(nc, maps, list(range(NC)))
    out = np.zeros((N, OUT), np.float32)
    for c in range(NC):
        out[c * NLOC:(c + 1) * NLOC] = res.results[c]["out"][0:NLOC]
    return out



# revision 43
# speedup vs baseline: 179.3813x; 1.0283x over previous
"""GATv2 2-layer GNN on 8 Trainium2 NeuronCores (Bass/Tile).

Sharding: dst-range (6250 dsts/core), edges dst-sorted into 49 windows of
128 dsts. Per-edge endpoint rows are fetched with dma_gather from f16 tables
(512B rows, int16 indices -> tables split at row 25088 into A/B halves).
The |att|-fold plus a +-1 sign mask turns sum_c att_c*lrelu(m_c) into
lrelu + masked reduce. Segment softmax skips max-subtraction (e stays in
[-7, 7] for this model; exp in f16 is safe). Segment sums run on the PE as
0/1-indicator matmuls accumulated into one PSUM tile per window. Layer 2
reuses the same index streams on its own tables. The program is
SPMD-uniform: per-window tile counts are maxed over cores; cores pad with
idx=0 / seg=128 slots which contribute zero.
"""
import sys

sys.path.insert(0, "/opt/trn_rl_repo")

import numpy as np

N = 50000
IN, HID, H, OUT = 128, 64, 4, 64
SLOPE = 0.2
NC = 8
NLOC = N // NC            # 6250
NPAD = 6272               # 49*128
NWIN = NPAD // 128        # 49
NROWS = NPAD * NC         # 50176
SPLIT = NROWS // 2        # 25088
GW = 2                    # windows per gather group
STREAM_MODE = False       # debug: replace gathers with sequential streams
CALL_CAP = 16             # max tiles (x128 idx) per dma_gather call
MODE = "full"             # full | stream | nogather | gather_only | noedge

_cache = {}


def _wrap16(stream):
    n = len(stream)
    a = np.zeros((16, n // 16), np.int16)
    a[np.arange(n) % 16, np.arange(n) // 16] = stream
    return np.tile(a, (8, 1))


def _host_metadata(edge_index):
    src = np.concatenate([np.asarray(edge_index[0], np.int64), np.arange(N)])
    dst = np.concatenate([np.asarray(edge_index[1], np.int64), np.arange(N)])
    srow = (src // NLOC) * NPAD + (src % NLOC)
    dcore = dst // NLOC
    dloc = dst % NLOC

    per_cw = [[None] * NWIN for _ in range(NC)]
    for c in range(NC):
        m = dcore == c
        sr, dl = srow[m], dloc[m]
        o = np.argsort(dl, kind="stable")
        sr, dl = sr[o], dl[o]
        wi = dl // 128
        for w in range(NWIN):
            ws = wi == w
            s_w, d_w = sr[ws], dl[ws] - w * 128
            a = s_w < SPLIT
            per_cw[c][w] = (s_w[a], d_w[a], s_w[~a] - SPLIT, d_w[~a])

    TA = [max((len(per_cw[c][w][0]) + 127) // 128 for c in range(NC))
          for w in range(NWIN)]
    TB = [max((len(per_cw[c][w][2]) + 127) // 128 for c in range(NC))
          for w in range(NWIN)]

    groups = []
    w = 0
    ti = 0
    while w < NWIN:
        ws = list(range(w, min(w + GW, NWIN)))
        na = sum(TA[x] for x in ws)
        nb = sum(TB[x] for x in ws)
        slots = [(x, "A", i) for x in ws for i in range(TA[x])] + \
                [(x, "B", i) for x in ws for i in range(TB[x])]
        t = len(slots)
        calls = []
        for kind, lo, hi in (("A", 0, na), ("B", na, na + nb)):
            s0 = lo
            while s0 < hi:
                n = min(CALL_CAP, hi - s0)
                calls.append((kind, s0, n))
                s0 += n
        groups.append(dict(ws=ws, na=na, nb=nb, slots=slots, calls=calls,
                           cols=sum(n * 8 for _, _, n in calls), ti=ti))
        ti += t
        w += GW
    tot_tiles = ti
    co0 = 0
    for g in groups:
        g["co0"] = co0
        co0 += g["cols"]

    seg_all = np.full((NC, 128, tot_tiles), 128, np.int16)
    idx_all = [np.zeros((NC, 128, g["cols"]), np.int16) for g in groups]
    for gi, g in enumerate(groups):
        nslot = len(g["slots"])
        for c in range(NC):
            slot_idx = np.zeros((nslot, 128), np.int64)
            for si, (w, kind, k) in enumerate(g["slots"]):
                sa, da, sb, db = per_cw[c][w]
                s_, d_ = (sa, da) if kind == "A" else (sb, db)
                iv = s_[k * 128:(k + 1) * 128]
                dv = d_[k * 128:(k + 1) * 128]
                n = len(iv)
                slot_idx[si, :n] = iv
                seg_all[c, :n, g["ti"] + si] = dv
            co = 0
            for kind, s0, ntc in g["calls"]:
                idx_all[gi][c, :, co:co + ntc * 8] = _wrap16(
                    slot_idx[s0:s0 + ntc].reshape(-1))
                co += ntc * 8
    return groups, seg_all, idx_all, tot_tiles


def _build(groups, tot_tiles, rep=1):
    import concourse.bacc as bacc
    import concourse.mybir as mybir
    import concourse.tile as tile
    from concourse import library_config

    f16, f32, i16 = mybir.dt.float16, mybir.dt.float32, mybir.dt.int16
    A = mybir.AluOpType

    nc = bacc.Bacc("TRN2", target_bir_lowering=False, debug=False,
                   num_devices=NC, num_swdge_queues=4)
    totcols = sum(g["cols"] for g in groups)
    d = {}
    for name, shape, dt in [
            ("xT", [128, NPAD], f16), ("W1", [128, 512], f16),
            ("bxr1", [128, 256], f16), ("W2", [128, 256], f16),
            ("bxr2", [128, 64], f16), ("sg1", [128, 256], f16),
            ("sg2", [128, 64], f16), ("rc1", [128, 256], f16),
            ("rc2", [128, 64], f32), ("ba1", [128, 256], f16),
            ("ba2", [128, 64], f32), ("iot", [128, 128], i16),
            ("ident", [128, 128], f16),
            ("idxall", [128, totcols], i16),
            ("seg", [128, tot_tiles], i16)]:
        d[name] = nc.dram_tensor(name, shape, dt, kind="ExternalInput")
    out_d = nc.dram_tensor("out", [NPAD, 64], f32, kind="ExternalOutput")

    with tile.TileContext(nc) as tc:
        with (
            tc.tile_pool(name="const", bufs=1) as cp,
            tc.tile_pool(name="dram", bufs=1, space="DRAM") as dp,
        ):
            nc.gpsimd.load_library(library_config.mlp)
            T = {"out_d": out_d, "dp": dp, "xT_d": d["xT"]}
            for name in d:
                if name == "xT":
                    continue
                tl = cp.tile(list(d[name].shape), d[name].dtype, tag=name)
                nc.sync.dma_start(tl[:], d[name][:, :])
                T[name] = tl

            for r in range(rep):
                _one_pass(nc, tc, mybir, groups, T)
    nc.compile()
    return nc


def _one_pass(nc, tc, mybir, groups, T):
    f16, f32 = mybir.dt.float16, mybir.dt.float32
    A = mybir.AluOpType
    dp = T["dp"]

    xl_loc = dp.tile([NPAD, 256], f16, tag="xl_loc")
    xl_full = dp.tile([NROWS, 256], f16, tag="xl_full",
                      addr_space="Shared")
    l2_loc = dp.tile([NPAD, 128], f16, tag="l2_loc")
    l2_full = dp.tile([NROWS, 128], f16, tag="l2_full",
                      addr_space="Shared")

    with tc.tile_pool(name="acc", bufs=1) as accp:
        # whole-pass SBUF residents: xr/r2 window rows, transposed h, and
        # the gather index streams (shared by both edge layers).
        xracc = accp.tile([128, NWIN, 256], f16)
        r2acc = accp.tile([128, NWIN, 64], f16)
        hTacc = accp.tile([128, 2, NPAD], f16)
        idxs = accp.tile([128, T["idxall"].shape[1]], mybir.dt.int16)
        nc.sync.dma_start(idxs[:], T["idxall"][:])
        T["idxs"] = idxs

        # ---- node phase 1: att-folded xl/xr tables for local nodes ----
        with (
            tc.tile_pool(name="n1", bufs=1) as n1,
            tc.tile_pool(name="n1p", bufs=2, space="PSUM") as n1p,
        ):
            vlacc = n1.tile([128, NWIN, 256], f16)
            xTs = n1.tile([128, NPAD], f16)
            nc.sync.dma_start(xTs[:], T["xT_d"][:, :])
            for t in range(NWIN):
                ps = n1p.tile([128, 512], f32, tag="ps")
                nc.tensor.matmul(ps[:], xTs[:, t * 128:(t + 1) * 128],
                                 T["W1"][:], start=True, stop=True,
                                 skip_group_check=True)
                nc.scalar.copy(vlacc[:, t, :], ps[:, 0:256])
                nc.vector.tensor_tensor(xracc[:, t, :], ps[:, 256:512],
                                        T["bxr1"][:], A.add)
            nc.sync.dma_start(
                xl_loc[:, :].rearrange("(t p) c -> p t c", p=128), vlacc[:])

        nc.gpsimd.collective_compute(
            "AllGather", A.bypass, replica_groups=[list(range(NC))],
            ins=[xl_loc[:].opt()], outs=[xl_full[:].opt()])

        # ---- edge phase 1 -> hT rows in SBUF ----
        if MODE != "noedge":
            with (
                tc.tile_pool(name="e1", bufs=2) as e1,
                tc.tile_pool(name="e1s", bufs=1) as e1s,
                tc.tile_pool(name="e1p", bufs=2, space="PSUM") as e1p,
                tc.tile_pool(name="e1px", bufs=2, space="PSUM") as e1px,
            ):
                _edge_layer(nc, tc, mybir, groups, T, e1, e1s, e1p, e1px,
                            layer=1, src_tab=xl_full, xr_acc=xracc,
                            h=hTacc)

            # ---- node phase 2: hl2/hr2 tables ----
            if MODE == "gather_only":
                return
            with (
                tc.tile_pool(name="n2", bufs=1) as n2,
                tc.tile_pool(name="n2p", bufs=2, space="PSUM") as n2p,
            ):
                v2acc = n2.tile([128, NWIN, 128], f16)
                for t in range(NWIN):
                    ps = n2p.tile([128, 128], f32, tag="ps2")
                    for k in range(2):
                        nc.tensor.matmul(
                            ps[:], hTacc[:, k, t * 128:(t + 1) * 128],
                            T["W2"][:, k * 128:(k + 1) * 128],
                            start=(k == 0), stop=(k == 1),
                            skip_group_check=True)
                    nc.vector.memset(v2acc[:, t, 64:128], 0.0)
                    nc.scalar.copy(v2acc[:, t, 0:64], ps[:, 0:64])
                    nc.vector.tensor_tensor(r2acc[:, t, :], ps[:, 64:128],
                                            T["bxr2"][:], A.add)
                nc.sync.dma_start(
                    l2_loc[:, :].rearrange("(t p) c -> p t c", p=128),
                    v2acc[:])

        nc.gpsimd.collective_compute(
            "AllGather", A.bypass, replica_groups=[list(range(NC))],
            ins=[l2_loc[:].opt()], outs=[l2_full[:].opt()])

        # ---- edge phase 2 -> output rows ----
        if MODE not in ("noedge", "gather_only"):
            with (
                tc.tile_pool(name="e2", bufs=2) as e2,
                tc.tile_pool(name="e2s", bufs=1) as e2s,
                tc.tile_pool(name="e2p", bufs=2, space="PSUM") as e2p,
                tc.tile_pool(name="e2px", bufs=2, space="PSUM") as e2px,
            ):
                _edge_layer(nc, tc, mybir, groups, T, e2, e2s, e2p, e2px,
                            layer=2, src_tab=l2_full, xr_acc=r2acc, h=None)


def _edge_layer(nc, tc, mybir, groups, T, pool, spool, ppool, xpool, layer,
                src_tab, xr_acc, h):
    f16, f32 = mybir.dt.float16, mybir.dt.float32
    A = mybir.AluOpType
    AF = mybir.ActivationFunctionType
    CH = 256 if layer == 1 else 128   # gathered row width (f16 elems)
    CV = 256 if layer == 1 else 64    # valid channels
    NH = 4 if layer == 1 else 1       # heads
    CPH = CV // NH                    # channels per head
    RW = NH + CV                      # [w | wfeat]
    tg = f"l{layer}"

    for gi, g in enumerate(groups):
        nt = len(g["slots"])
        idxs = T["idxs"]
        xlg = pool.tile([128, nt, CH], f16, tag=tg + "xl")
        co = g["co0"]
        for ci, (kind, s0, ntc) in enumerate(g["calls"]):
            src = src_tab[0:SPLIT, :] if kind == "A" else src_tab[SPLIT:NROWS, :]
            if MODE == "nogather":
                nc.vector.memset(xlg[:, s0:s0 + ntc, 0:8], 0.0)
            elif STREAM_MODE or MODE == "stream":
                r0 = (g["ti"] * 64) % 4096
                nc.sync.dma_start(
                    xlg[:, s0:s0 + ntc, :],
                    src[r0:r0 + ntc * 128, :].rearrange(
                        "(t p) c -> p t c", p=128))
            else:
                nc.gpsimd.dma_gather(
                    xlg[:, s0:s0 + ntc, :], src, idxs[:, co:co + ntc * 8],
                    ntc * 128, ntc * 128, CH, single_packet=False,
                    queue_num=(gi * 2 + ci) % 4)
            co += ntc * 8

        ind = pool.tile([128, nt, 128], f16, tag=tg + "ind")
        iot_b = T["iot"][:].unsqueeze(1).broadcast_to([128, nt, 128])
        seg_b = T["seg"][:, g["ti"]:g["ti"] + nt].unsqueeze(2)\
            .broadcast_to([128, nt, 128])
        nc.vector.tensor_tensor(ind[:], iot_b, seg_b, A.is_equal)

        # per-edge xr rows via transposed-indicator matmuls (no DMA gather);
        # transposes/matmuls land in quad-batched PSUM tiles so one scalar
        # copy moves four tiles' worth.
        indT = pool.tile([128, nt, 128], f16, tag=tg + "indT")
        xr_sb = pool.tile([128, nt, CV], f16, tag=tg + "xrsb")
        for si, (w, kind, k) in enumerate(g["slots"]):
            psT = xpool.tile([128, 128], f16, tag=tg + "psT",
                             name=f"{tg}psT")
            nc.tensor.transpose(psT[:], ind[:, si, :], T["ident"][:])
            nc.scalar.copy(indT[:, si, :], psT[:])
            xre = xpool.tile([128, CV], f32, tag=tg + "xre",
                             name=f"{tg}xre")
            nc.tensor.matmul(xre[:], indT[:, si, :], xr_acc[:, w, 0:CV],
                             start=True, stop=True, skip_group_check=True)
            nc.scalar.copy(xr_sb[:, si, :], xre[:])

        if MODE == "gather_only":
            return

        xl3 = xlg[:] if layer == 1 else xlg[:, :, 0:64]
        m = spool.tile([128, nt, CV], f16, tag=tg + "m")
        nc.vector.tensor_tensor(m[:], xl3, xr_sb[:], A.add)
        t_ = spool.tile([128, nt, CV], f16, tag=tg + "t")
        nc.vector.scalar_tensor_tensor(t_[:], m[:], SLOPE, m[:],
                                       A.mult, A.max)
        sgn = (T["sg1"] if layer == 1 else T["sg2"])[:].unsqueeze(1)\
            .broadcast_to([128, nt, CV])
        nc.vector.tensor_tensor(m[:], t_[:], sgn, A.mult)
        e = spool.tile([128, nt, NH], f32, tag=tg + "e")
        nc.vector.reduce_sum(
            e[:], m[:].rearrange("p t (h c) -> p t h c", h=NH),
            axis=mybir.AxisListType.X)
        rhs = pool.tile([128, nt, RW], f16, tag=tg + "rhs")
        nc.scalar.activation(rhs[:, :, 0:NH], e[:], AF.Exp)
        wb = rhs[:, :, 0:NH].unsqueeze(3).broadcast_to([128, nt, NH, CPH])
        xl4 = xl3.rearrange("p t (h c) -> p t h c", h=NH)
        nc.vector.tensor_tensor(
            rhs[:, :, NH:RW].rearrange("p t (h c) -> p t h c", h=NH),
            xl4, wb, A.mult)

        pstiles = {}
        last_slot = {}
        for si, (w, kind, k) in enumerate(g["slots"]):
            last_slot[w] = si
        for si, (w, kind, k) in enumerate(g["slots"]):
            st = w not in pstiles
            if st:
                pstiles[w] = ppool.tile([128, RW], f32, tag=tg + "ps",
                                        name=f"{tg}ps{w}")
            nc.tensor.matmul(pstiles[w][:], ind[:, si, :], rhs[:, si, :],
                             start=st, stop=(si == last_slot[w]),
                             skip_group_check=True)

        for w in g["ws"]:
            ps = pstiles[w]
            if layer == 1:
                rec = spool.tile([128, 4], f32, tag=tg + "rec")
                nc.vector.reciprocal(rec[:], ps[:, 0:4])
                u = spool.tile([128, 256], f16, tag=tg + "u")
                nc.vector.tensor_tensor(
                    u[:].rearrange("p (h c) -> p h c", h=4),
                    ps[:, 4:260].rearrange("p (h c) -> p h c", h=4),
                    rec[:].unsqueeze(2).broadcast_to([128, 4, 64]), A.mult)
                nc.vector.tensor_tensor(u[:], u[:], T["rc1"][:], A.mult)
                nc.vector.tensor_tensor(u[:], u[:], T["ba1"][:], A.add)
                lo = spool.tile([128, 256], f16, tag=tg + "lo")
                nc.vector.tensor_scalar_min(lo[:], u[:], 0.0)
                ex = spool.tile([128, 256], f16, tag=tg + "ex")
                nc.scalar.activation(ex[:], lo[:], AF.Exp)
                nc.vector.tensor_scalar_max(u[:], u[:], 0.0)
                hrow = pool.tile([128, 256], f16, tag=tg + "hrow")
                nc.vector.scalar_tensor_tensor(
                    hrow[:], ex[:], -1.0, u[:], A.add, A.add)
                for k in range(2):
                    psH = xpool.tile([128, 128], f16, tag=tg + "psT",
                                     name=f"{tg}psH")
                    nc.tensor.transpose(psH[:],
                                        hrow[:, k * 128:(k + 1) * 128],
                                        T["ident"][:])
                    nc.scalar.copy(h[:, k, w * 128:(w + 1) * 128], psH[:])
            else:
                rec = spool.tile([128, 1], f32, tag=tg + "rec2")
                nc.vector.reciprocal(rec[:], ps[:, 0:1])
                u = spool.tile([128, 64], f32, tag=tg + "u2")
                nc.vector.scalar_tensor_tensor(
                    u[:], ps[:, 1:65], rec[:], T["rc2"][:], A.mult, A.mult)
                nc.vector.tensor_tensor(u[:], u[:], T["ba2"][:], A.add)
                nc.sync.dma_start(T["out_d"][w * 128:(w + 1) * 128, :], u[:])


def _host_prep(inputs):
    att1 = np.asarray(inputs["att1"], np.float64)
    att2 = np.asarray(inputs["att2"], np.float64)[0]
    f1 = np.maximum(np.abs(att1.reshape(-1)), 1e-30)
    s1 = np.where(att1.reshape(-1) >= 0, 1.0, -1.0)
    f2 = np.maximum(np.abs(att2), 1e-30)
    s2 = np.where(att2 >= 0, 1.0, -1.0)

    W1 = np.concatenate([np.asarray(inputs["Wl1"], np.float64) * f1,
                         np.asarray(inputs["Wr1"], np.float64) * f1],
                        1)
    b1 = np.concatenate([np.asarray(inputs["bl1"], np.float64) * f1,
                         np.asarray(inputs["br1"], np.float64) * f1])
    W2c = np.concatenate([np.asarray(inputs["Wl2"], np.float64) * f2,
                          np.asarray(inputs["Wr2"], np.float64) * f2],
                         1)                      # [256, 128]
    W2 = np.concatenate([W2c[0:128], W2c[128:256]], 1)  # [128, 256] 2 chunks
    b2 = np.concatenate([np.asarray(inputs["bl2"], np.float64) * f2,
                         np.asarray(inputs["br2"], np.float64) * f2])

    bxr1 = (b1[0:256] + b1[256:512]).astype(np.float16)
    bxr2 = (b2[0:64] + b2[64:128]).astype(np.float16)
    bl1 = np.asarray(inputs["bl1"], np.float64)
    bl2 = np.asarray(inputs["bl2"], np.float64)
    com = dict(
        W1=W1.astype(np.float16),
        bxr1=np.tile(bxr1, (128, 1)),
        W2=W2.astype(np.float16),
        bxr2=np.tile(bxr2, (128, 1)),
        sg1=np.tile(s1.astype(np.float16), (128, 1)),
        sg2=np.tile(s2.astype(np.float16), (128, 1)),
        rc1=np.tile((1.0 / f1).astype(np.float16), (128, 1)),
        rc2=np.tile((1.0 / f2).astype(np.float32), (128, 1)),
        ba1=np.tile((np.asarray(inputs["bias1"], np.float64) + bl1)
                    .astype(np.float16), (128, 1)),
        ba2=np.tile((np.asarray(inputs["bias2"], np.float64) + bl2)
                    .astype(np.float32), (128, 1)),
        iot=np.tile(np.arange(128, dtype=np.int16), (128, 1)),
        ident=np.eye(128, dtype=np.float16),
    )
    x = np.asarray(inputs["x"], np.float32)
    xTs = []
    for c in range(NC):
        xt = np.zeros((128, NPAD), np.float16)
        xt[:, 0:NLOC] = x[c * NLOC:(c + 1) * NLOC].T.astype(np.float16)
        xTs.append(xt)
    return com, xTs


def _get_built(edge_index, rep=1):
    key = (hash(np.asarray(edge_index).tobytes()), rep, MODE)
    if key not in _cache:
        groups, seg_all, idx_all, tot_tiles = _host_metadata(edge_index)
        nc = _build(groups, tot_tiles, rep=rep)
        _cache[key] = (groups, seg_all, idx_all, nc)
    return _cache[key]


def make_maps(inputs, seg_all, idx_all):
    com, xTs = _host_prep(inputs)
    maps = []
    for c in range(NC):
        m = dict(com)
        m["xT"] = xTs[c]
        m["seg"] = seg_all[c]
        m["idxall"] = np.concatenate([a[c] for a in idx_all], axis=1)
        maps.append(m)
    return maps


def kernel(**inputs):
    from concourse.bass_utils import run_bass_kernel_spmd

    groups, seg_all, idx_all, nc = _get_built(inputs["edge_index"])
    maps = make_maps(inputs, seg_all, idx_all)
    res = run_bass_kernel_spmd(nc, maps, list(range(NC)))
    out = np.zeros((N, OUT), np.float32)
    for c in range(NC):
        out[c * NLOC:(c + 1) * NLOC] = res.results[c]["out"][0:NLOC]
    return out

